# revision 23
# baseline (speedup 1.0000x reference)
"""Trainium2 Bass kernel for a Bayesian MLP (local reparameterization trick).

Reference computation (per sample s of S=10):
    h1 = leaky_relu(x @ W1m + sqrt(x^2 @ W1v + 1e-12) * eps1_s)         [B, 512]
    h2 = leaky_relu(h1a @ W2m + sqrt(h1a^2 @ W2v + 1e-12) * eps2_s)     (h1a = [h1, 1])
    h3 = leaky_relu(h2a @ W3m + sqrt(h2a^2 @ W3v + 1e-12) * eps3_s)
    out = log_softmax(h3a @ W4m + sqrt(h3a^2 @ W4v + 1e-12) * eps4_s)   [B, 10]

Distribution: data-parallel over the batch axis, B=2048 -> 8 cores x 256 rows.
Small variational parameters replicated on every core.

Fast path (used when every a*_scale array is a constant fill, which holds for
the reference setup where scale = 0.1 * ones):
    x^2 @ (c*ones) = c * ||x||^2  -> the whole variance path collapses to a
    rank-1 partition-sum matmul per layer + one sqrt on a [1, fd] row.
  * activations [feat on 128 partitions, (sample,batch) free], bf16 matmuls
  * sigma*eps is accumulated into the mean PSUM via an identity matmul, so a
    single ACT Prelu (with the folded mean-bias) reads PSUM once per tile
  * per-engine balance: DVE does sig*e products and h^2 squares, ACT does
    prelus/sqrts, Pool (gpsimd) does PSUM->SBUF sigma/u4 copies + L1 squares
  * log-softmax deferred to a final phase (exp/ln table loads once)

General path: the original baseline program (full variance matmuls, f32r).
"""

import sys
import os

for _p in ("/opt/trn_rl_repo",):
    if _p not in sys.path and os.path.isdir(_p):
        sys.path.insert(0, _p)

import numpy as np
import ml_dtypes

import concourse.bass as bass
import concourse.bacc as bacc
import concourse.mybir as mybir
from concourse import tile
from concourse.bass_utils import run_bass_kernel_spmd

F32 = mybir.dt.float32
F32R = mybir.dt.float32r
BF16 = mybir.dt.bfloat16
F8E4 = mybir.dt.float8e4
AF = mybir.ActivationFunctionType
ALU = mybir.AluOpType

B, D_IN, H, C, S = 2048, 784, 512, 10, 10
N_CORES = 8
BL = B // N_CORES            # 256 rows per core
KPAD = 896                   # 784 padded to 7*128
K1 = KPAD // 128             # 7 k-chunks for layer 1
KH = H // 128                # 4 k-chunks for hidden layers
FO = H // 128                # 4 output-feature chunks for hidden layers

bf = ml_dtypes.bfloat16


# --------------------------------------------------------------------------
# Fast path: uniform scale arrays -> rank-1 variance
# --------------------------------------------------------------------------

def build_program_fast(c1, c2, c3, c4, bl=BL, n_pairs=S // 2):
    """Per-core program exploiting sigma_l^2 = c_l * (||h||^2 [+ 1]).

    sqrt is computed as exp(0.5*ln(.)) so the whole kernel runs inside the
    single natural_log_exp ACT table set; log-softmax interleaves per pair.
    sigma rows broadcast across partitions via gpsimd partition_broadcast.
    """
    fd = 2 * bl              # 512 free dim per sample-pair
    nc = bacc.Bacc("TRN2", target_bir_lowering=False, debug=False)

    # ---- DRAM I/O (per core) ----
    xT_d = nc.dram_tensor("xT", [K1, 128, bl], F8E4, kind="ExternalInput")
    x2T_d = nc.dram_tensor("x2T", [K1, 128, bl], F8E4, kind="ExternalInput")
    w1m_d = nc.dram_tensor("w1m", [K1, 128, H], F8E4, kind="ExternalInput")
    w2m_d = nc.dram_tensor("w2m", [KH, 128, H], BF16, kind="ExternalInput")
    w3m_d = nc.dram_tensor("w3m", [KH, 128, H], BF16, kind="ExternalInput")
    w4m_d = nc.dram_tensor("w4m", [128, KH * C], BF16, kind="ExternalInput")
    b2mP_d = nc.dram_tensor("b2mP", [128, FO], F32, kind="ExternalInput")
    b3mP_d = nc.dram_tensor("b3mP", [128, FO], F32, kind="ExternalInput")
    b4m_d = nc.dram_tensor("b4m", [1, C], BF16, kind="ExternalInput")
    e1_d = nc.dram_tensor("e1", [n_pairs, FO, 128, fd], BF16, kind="ExternalInput")
    e2_d = nc.dram_tensor("e2", [n_pairs, FO, 128, fd], BF16, kind="ExternalInput")
    e3_d = nc.dram_tensor("e3", [n_pairs, FO, 128, fd], BF16, kind="ExternalInput")
    e4_d = nc.dram_tensor("e4", [n_pairs, C, fd], BF16, kind="ExternalInput")
    eye128_d = nc.dram_tensor("eye128", [128, 128], BF16, kind="ExternalInput")
    eye10_d = nc.dram_tensor("eye10", [C, C], BF16, kind="ExternalInput")
    o_1x10_d = nc.dram_tensor("o_1x10", [1, C], BF16, kind="ExternalInput")
    n_1x10_d = nc.dram_tensor("n_1x10", [1, C], BF16, kind="ExternalInput")
    o_sq_d = nc.dram_tensor("o_sq", [128, 128], BF16, kind="ExternalInput")
    o_128x10_d = nc.dram_tensor("o_128x10", [128, C], BF16, kind="ExternalInput")
    o_128x1_d = nc.dram_tensor("o_128x1", [128, 1], BF16, kind="ExternalInput")
    o_10x1_d = nc.dram_tensor("o_10x1", [C, 1], BF16, kind="ExternalInput")
    ones_row_d = nc.dram_tensor("ones_row", [1, fd], BF16, kind="ExternalInput")
    out_d = nc.dram_tensor("out", [n_pairs, C, fd], F32, kind="ExternalOutput")

    mm = nc.tensor.matmul

    with tile.TileContext(nc) as tc:
        with (
            tc.tile_pool(name="wp", bufs=1) as wp,
        ):
            # persistent tiles
            w2m_t = [wp.tile([128, H], BF16, tag=f"w2m{k}", name=f"w2m{k}") for k in range(KH)]
            w3m_t = [wp.tile([128, H], BF16, tag=f"w3m{k}", name=f"w3m{k}") for k in range(KH)]
            w4m_t = wp.tile([128, KH * C], BF16, tag="w4m", name="w4m")
            b2mP_t = wp.tile([128, FO], F32, tag="b2mP", name="b2mP")
            b3mP_t = wp.tile([128, FO], F32, tag="b3mP", name="b3mP")
            b4m_t = wp.tile([1, C], BF16, tag="b4m", name="b4m")
            eye128_t = wp.tile([128, 128], BF16, tag="eye128", name="eye128")
            eye10_t = wp.tile([C, C], BF16, tag="eye10", name="eye10")
            o_1x10 = wp.tile([1, C], BF16, tag="o_1x10", name="o_1x10")
            n_1x10 = wp.tile([1, C], BF16, tag="n_1x10", name="n_1x10")
            o_sq = wp.tile([128, 128], BF16, tag="o_sq", name="o_sq")
            o_128x10 = wp.tile([128, C], BF16, tag="o_128x10", name="o_128x10")
            o_128x1 = wp.tile([128, 1], BF16, tag="o_128x1", name="o_128x1")
            o_10x1 = wp.tile([C, 1], BF16, tag="o_10x1", name="o_10x1")
            ones_row = wp.tile([1, fd], BF16, tag="ones_row", name="ones_row")
            mu1_t = wp.tile([128, FO * bl], BF16, tag="mu1", name="mu1")
            sig1b_t = wp.tile([128, bl], BF16, tag="sig1b", name="sig1b")
            z128_t = wp.tile([128, 1], F32, tag="z128", name="z128")
            zC_t = wp.tile([C, 1], F32, tag="zC", name="zC")
            z1_t = wp.tile([1, 1], F32, tag="z1", name="z1")
            cb_t = {}
            for nm, cv, npart in (("c2", c2, 128), ("c3", c3, 128),
                                  ("c4", c4, C)):
                cb_t[nm] = wp.tile([npart, 1], F32, tag=f"cb_{nm}",
                                   name=f"cb_{nm}")
                nc.vector.memset(cb_t[nm][:], float(cv))
            nc.vector.memset(z128_t[:], 0.0)
            nc.vector.memset(zC_t[:], 0.0)
            nc.vector.memset(z1_t[:], 0.0)

            # phase-A-critical small constants first
            nc.sync.dma_start(o_128x1[:], o_128x1_d[:])
            nc.sync.dma_start(o_sq[:], o_sq_d[:])

            # ---------- Phase A: layer-1 mu / sigma, sample-independent ----------
            with (
                tc.tile_pool(name="ap", bufs=1) as ap,
                tc.tile_pool(name="psA", bufs=1, space="PSUM") as psA,
            ):
                w1m_t = [ap.tile([128, H], F8E4, tag=f"w1m{k}", name=f"w1m{k}") for k in range(K1)]
                xT_t = [ap.tile([128, bl], F8E4, tag=f"xT{k}", name=f"xT{k}") for k in range(K1)]
                x2T_t = [ap.tile([128, bl], F8E4, tag=f"x2T{k}", name=f"x2T{k}") for k in range(K1)]
                o_sq8 = ap.tile([128, 128], F8E4, tag="o_sq8", name="o_sq8")
                nc.vector.memset(o_sq8[:], 1.0)
                # ~3.5us of dummy matmuls on memset tiles: wakes the PE HAM
                # clock gate (idle default is 1.2 GHz; sustained activity
                # unlocks 2.4 GHz) before the DMA-dependent real work lands.
                warm_rhs = ap.tile([128, fd], F8E4, tag="warm", name="warm")
                nc.vector.memset(warm_rhs[:], 0.0)
                warm_ps = psA.tile([128, fd], F32, tag="psA_w", name="psA_w")
                for i in range(9):
                    mm(warm_ps[:], o_sq8[:], warm_rhs[:], start=True, stop=True)
                for k in range(K1):
                    nc.sync.dma_start(x2T_t[k][:], x2T_d[k])
                for k in range(K1):
                    nc.sync.dma_start(xT_t[k][:], xT_d[k])
                    nc.sync.dma_start(w1m_t[k][:], w1m_d[k])

                # sigma1 first (it gates pair-0's L1 chain), then mu fo-major
                # so mu1[fo0] lands early and L1(0) overlaps the rest
                mu_ps = [psA.tile([128, bl], F32, tag=f"psA_mu{fo}",
                                  name=f"psA_mu{fo}") for fo in range(FO)]
                s1_ps = psA.tile([128, bl], F32, tag="psA_s1", name="psA_s1")
                for k in range(K1):
                    mm(s1_ps[:], o_sq8[:], x2T_t[k][:],
                       start=(k == 0), stop=(k == K1 - 1))
                lnv1 = ap.tile([128, bl], F32, tag="lnv1", name="lnv1")
                nc.scalar.activation(lnv1[:], s1_ps[:], AF.Ln, scale=float(c1),
                                     bias=z128_t[:])
                nc.scalar.activation(sig1b_t[:], lnv1[:], AF.Exp, scale=0.5,
                                     bias=z128_t[:])
                for fo in range(FO):
                    for k in range(K1):
                        mm(mu_ps[fo][:], w1m_t[k][:, fo * 128:(fo + 1) * 128],
                           xT_t[k][:], start=(k == 0), stop=(k == K1 - 1))
                    nc.vector.tensor_scalar_mul(
                        mu1_t[:, fo * bl:(fo + 1) * bl], mu_ps[fo][:],
                        1.0 / 16.0)

            # ---------- Phase B: per sample-pair, layers 1-4 + softmax ----------
            with (
                tc.tile_pool(name="ep", bufs=3) as ep,
                tc.tile_pool(name="hp", bufs=2) as hp,
                tc.tile_pool(name="tp", bufs=8) as tp,
                tc.tile_pool(name="sgp", bufs=2) as sgp,
                tc.tile_pool(name="psU", bufs=1, space="PSUM") as psU,
                tc.tile_pool(name="psV", bufs=1, space="PSUM") as psV,
            ):
                def dma_eps(p, with_weights=None):
                    e_t = {}
                    for nm, e_d in (("e1", e1_d), ("e2", e2_d), ("e3", e3_d)):
                        e_t[nm] = [ep.tile([128, fd], BF16, tag=f"{nm}_{k}",
                                           name=f"{nm}_{k}") for k in range(FO)]
                        for k in range(FO):
                            nc.sync.dma_start(e_t[nm][k][:], e_d[p, k])
                    e_t["e4"] = ep.tile([C, fd], BF16, tag="e4", name="e4")
                    nc.sync.dma_start(e_t["e4"][:], e4_d[p])
                    return e_t

                def emit_L1(eps):
                    """u1 = mu1 + sig1*e1 entirely in SBUF (no PE)."""
                    h_t, hq_t = [], []
                    for fo in range(FO):
                        sl = slice(fo * bl, (fo + 1) * bl)
                        sig_b = (sig1b_t[:].unsqueeze(1)
                                 .broadcast_to((128, 2, bl)))
                        mu_b = (mu1_t[:, sl].unsqueeze(1)
                                .broadcast_to((128, 2, bl)))
                        t_t = tp.tile([128, fd], BF16, tag="t1", name="t1", bufs=4)
                        nc.vector.tensor_tensor(
                            t_t[:].rearrange("p (s n) -> p s n", s=2),
                            eps["e1"][fo][:].rearrange("p (s n) -> p s n", s=2),
                            sig_b, ALU.mult)
                        u_t = tp.tile([128, fd], BF16, tag="u1", name="u1", bufs=4)
                        nc.vector.tensor_tensor(
                            u_t[:].rearrange("p (s n) -> p s n", s=2),
                            t_t[:].rearrange("p (s n) -> p s n", s=2),
                            mu_b, ALU.add)
                        h = hp.tile([128, fd], BF16, tag=f"h1_{fo}", name=f"h1_{fo}")
                        nc.scalar.activation(h[:], u_t[:], AF.Prelu,
                                             bias=z128_t[:], alpha=0.01)
                        hq = hp.tile([128, fd], BF16, tag=f"h1q_{fo}",
                                     name=f"h1q_{fo}")
                        if fo % 2 == 0:
                            nc.vector.tensor_tensor(hq[:], h[:], h[:], ALU.mult)
                        else:
                            nc.gpsimd.tensor_mul(hq[:], h[:], h[:])
                        h_t.append(h)
                        hq_t.append(hq)
                    return h_t, hq_t

                def emit_varones(hq_t, lhs, npart):
                    """||h||^2 summed over partitions AND broadcast to npart
                    partitions in one go (all-ones stationary matrix)."""
                    v_ps = psV.tile([128, fd], F32, tag="var", name="var", bufs=2)
                    for k in range(KH):
                        mm(v_ps[0:npart, :], lhs[:, 0:npart], hq_t[k][:],
                           start=(k == 0), stop=(k == KH - 1))
                    return v_ps

                def sigma_tail(v_ps, c, cb, npart, tag):
                    """sigma = exp(0.5*ln(c*S + c)) on the broadcast tile."""
                    lnv = tp.tile([npart, fd], F32, tag=f"lnv{tag}",
                                  name=f"lnv{tag}", bufs=2)
                    nc.scalar.activation(lnv[:], v_ps[0:npart, :], AF.Ln,
                                         scale=float(c), bias=cb[:])
                    sigb = sgp.tile([npart, fd], BF16, tag=f"sigb{tag}",
                                    name=f"sigb{tag}")
                    zb = z128_t if npart == 128 else zC_t
                    nc.scalar.activation(sigb[:], lnv[:], AF.Exp,
                                         scale=0.5, bias=zb[:])
                    return sigb

                def emit_hidden(eps_l, hin, sigb, wm_t, bmP_t, htag, vnext):
                    """One hidden layer; accumulates next layer's ||h||^2 into
                    vnext as each hq chunk completes (keeps the sigma chain of
                    the NEXT layer off the PE critical path)."""
                    hout, houtq = [], []
                    for fo in range(FO):
                        u_ps = psU.tile([128, fd], F32, tag=f"u{fo}", name=f"u{fo}")
                        for k in range(KH):
                            mm(u_ps[:], wm_t[k][:, fo * 128:(fo + 1) * 128],
                               hin[k][:], start=(k == 0), stop=False)
                        t_t = tp.tile([128, fd], BF16, tag="t", name="t", bufs=4)
                        nc.vector.tensor_tensor(t_t[:], eps_l[fo][:], sigb[:],
                                                ALU.mult)
                        mm(u_ps[:], eye128_t[:], t_t[:], start=False, stop=True)
                        h = hp.tile([128, fd], BF16, tag=f"{htag}_{fo}",
                                    name=f"{htag}_{fo}")
                        nc.scalar.activation(h[:], u_ps[:], AF.Prelu,
                                             bias=bmP_t[:, fo:fo + 1], alpha=0.01)
                        hq = hp.tile([128, fd], BF16, tag=f"{htag}q_{fo}",
                                     name=f"{htag}q_{fo}")
                        nc.vector.tensor_tensor(hq[:], h[:], h[:], ALU.mult)
                        if vnext is not None:
                            vn, vlhs, vnp = vnext
                            mm(vn[0:vnp, :], vlhs[:, 0:vnp], hq[:],
                               start=(fo == 0), stop=(fo == FO - 1))
                        hout.append(h)
                        houtq.append(hq)
                    return hout, houtq

                def emit_L4(p, h3_t, sigb4, eps):
                    t4 = tp.tile([C, fd], BF16, tag="t4", name="t4", bufs=2)
                    nc.vector.tensor_tensor(t4[:], eps["e4"][:], sigb4[:], ALU.mult)
                    u4_ps = psU.tile([C, fd], F32, tag="u4p", name="u4p")
                    for k in range(KH):
                        mm(u4_ps[:], w4m_t[:, k * C:(k + 1) * C], h3_t[k][:],
                           start=(k == 0), stop=False)
                    mm(u4_ps[:], b4m_t[:], ones_row[:], start=False, stop=False)
                    mm(u4_ps[:], eye10_t[:], t4[:], start=False, stop=True)
                    return u4_ps

                def emit_softmax(p, u4_ps):
                    # log-softmax in place on the u4 PSUM bank
                    e_t = tp.tile([C, fd], BF16, tag="expt", name="expt", bufs=2)
                    nc.scalar.activation(e_t[:], u4_ps[:], AF.Exp, bias=zC_t[:])
                    s_ps = psV.tile([128, fd], F32, tag="var", name="var", bufs=2)
                    mm(s_ps[0:1, :], o_10x1[:], e_t[:], start=True, stop=True)
                    lse_t = tp.tile([1, fd], BF16, tag="lse", name="lse", bufs=2)
                    nc.scalar.activation(lse_t[:], s_ps[0:1, :], AF.Ln,
                                         bias=z1_t[:])
                    mm(u4_ps[:], n_1x10[:], lse_t[:], start=False, stop=True,
                       skip_group_check=True)
                    o_t = tp.tile([C, fd], F32, tag="oct", name="oct", bufs=2)
                    nc.vector.tensor_copy(o_t[:], u4_ps[:])
                    nc.sync.dma_start(out_d[p], o_t[:])

                # prologue: eps(0) first in the DMA queue, then weights, eps(1)
                eps_cur = dma_eps(0)
                for k in range(KH):
                    nc.sync.dma_start(w2m_t[k][:], w2m_d[k])
                nc.sync.dma_start(b2mP_t[:], b2mP_d[:])
                nc.sync.dma_start(eye128_t[:], eye128_d[:])
                eps_next = dma_eps(1) if n_pairs > 1 else None
                for k in range(KH):
                    nc.sync.dma_start(w3m_t[k][:], w3m_d[k])
                nc.sync.dma_start(b3mP_t[:], b3mP_d[:])
                nc.sync.dma_start(w4m_t[:], w4m_d[:])
                nc.sync.dma_start(b4m_t[:], b4m_d[:])
                nc.sync.dma_start(eye10_t[:], eye10_d[:])
                nc.sync.dma_start(o_1x10[:], o_1x10_d[:])
                nc.sync.dma_start(n_1x10[:], n_1x10_d[:])
                nc.sync.dma_start(o_10x1[:], o_10x1_d[:])
                nc.sync.dma_start(o_128x10[:], o_128x10_d[:])
                nc.sync.dma_start(ones_row[:], ones_row_d[:])

                h1_cur = emit_L1(eps_cur)
                sigb2_cur = sigma_tail(emit_varones(h1_cur[1], o_sq, 128), c2,
                                       cb_t["c2"], 128, "2")
                u4_prev = None
                for p in range(n_pairs):
                    h1_t, hq1_t = h1_cur
                    v3_ps = psV.tile([128, fd], F32, tag="var", name="var", bufs=2)
                    h2_t, hq2_t = emit_hidden(eps_cur["e2"], h1_t, sigb2_cur,
                                              w2m_t, b2mP_t, "h2",
                                              (v3_ps, o_sq, 128))
                    sigb3 = sigma_tail(v3_ps, c3, cb_t["c3"], 128, "3")
                    eps_pf = None
                    if p + 2 < n_pairs:
                        eps_pf = dma_eps(p + 2)
                    h1_next = emit_L1(eps_next) if eps_next is not None else None
                    if u4_prev is not None:
                        emit_softmax(p - 1, u4_prev)
                    v4_ps = psV.tile([128, fd], F32, tag="var", name="var", bufs=2)
                    h3_t, hq3_t = emit_hidden(eps_cur["e3"], h2_t, sigb3,
                                              w3m_t, b3mP_t, "h3",
                                              (v4_ps, o_128x10, C))
                    sigb4 = sigma_tail(v4_ps, c4, cb_t["c4"], C, "4")
                    if h1_next is not None:
                        sigb2_cur = sigma_tail(emit_varones(h1_next[1], o_sq,
                                                            128), c2,
                                               cb_t["c2"], 128, "2")
                    u4_prev = emit_L4(p, h3_t, sigb4, eps_cur)
                    h1_cur = h1_next
                    eps_cur, eps_next = eps_next, eps_pf
                emit_softmax(n_pairs - 1, u4_prev)

    import concourse.bacc as _bacc_mod
    _orig_gat = _bacc_mod.get_activation_tables

    def _pinned_tables(arch):
        tabs = _orig_gat(arch)
        keep = "natural_log_exp_and_others"
        return {nm: (fns if nm == keep else set()) for nm, fns in tabs.items()}

    _bacc_mod.get_activation_tables = _pinned_tables
    try:
        nc.compile()
    finally:
        _bacc_mod.get_activation_tables = _orig_gat
    return nc


def prepare_core_inputs_fast(inputs, bl=BL, n_pairs=S // 2):
    ns = 2 * n_pairs
    fd = 2 * bl
    f = np.float32
    x = np.asarray(inputs["inputs"], dtype=f)

    def padK(a):
        out = np.zeros((KPAD, a.shape[1]), dtype=f)
        out[:D_IN] = a
        return out

    f8 = ml_dtypes.float8_e4m3fn
    w1m = (padK(np.asarray(inputs["a1_mean"], f)) * np.float32(16.0)) \
        .reshape(K1, 128, H).astype(f8)

    def hidden_w(mean):
        m = np.asarray(mean, f)
        wm = np.ascontiguousarray(m[:H].reshape(KH, 128, H).astype(bf))
        bmP = np.ascontiguousarray(m[H].reshape(FO, 128).T.astype(f))
        return wm, bmP

    w2m, b2mP = hidden_w(inputs["a2_mean"])
    w3m, b3mP = hidden_w(inputs["a3_mean"])

    m4 = np.asarray(inputs["a4_mean"], f)
    w4m = np.ascontiguousarray(m4[:H].reshape(KH, 128, C).transpose(1, 0, 2)
                               .reshape(128, KH * C).astype(bf))
    b4m = np.ascontiguousarray(m4[H].reshape(1, C).astype(bf))

    shared = dict(
        w1m=w1m, w2m=w2m, w3m=w3m, w4m=w4m,
        b2mP=b2mP, b3mP=b3mP, b4m=b4m,
        eye128=np.eye(128, dtype=bf),
        eye10=np.eye(C, dtype=bf),
        o_1x10=np.ones((1, C), dtype=bf),
        o_128x1=np.ones((128, 1), dtype=bf),
        o_sq=np.ones((128, 128), dtype=bf),
        o_128x10=np.ones((128, C), dtype=bf),
        n_1x10=np.full((1, C), -1.0, dtype=bf),
        o_10x1=np.ones((C, 1), dtype=bf),
        ones_row=np.ones((1, fd), dtype=bf),
    )

    eps1 = np.asarray(inputs["eps1"], f)
    eps2 = np.asarray(inputs["eps2"], f)
    eps3 = np.asarray(inputs["eps3"], f)
    eps4 = np.asarray(inputs["eps4"], f)

    def eT(e, b0):
        ec = e[:ns, b0:b0 + bl, :]
        return np.ascontiguousarray(
            ec.reshape(n_pairs, 2, bl, FO, 128).transpose(0, 3, 4, 1, 2)
            .reshape(n_pairs, FO, 128, fd).astype(bf))

    def e4T(e, b0):
        ec = e[:ns, b0:b0 + bl, :]
        return np.ascontiguousarray(
            ec.reshape(n_pairs, 2, bl, C).transpose(0, 3, 1, 2)
            .reshape(n_pairs, C, fd).astype(bf))

    in_maps = []
    for i in range(N_CORES):
        b0 = i * bl
        xT = np.zeros((KPAD, bl), dtype=f)
        xT[:D_IN] = x[b0:b0 + bl].T
        f8 = ml_dtypes.float8_e4m3fn
        m = dict(shared)
        m["xT"] = np.ascontiguousarray(xT.reshape(K1, 128, bl).astype(f8))
        m["x2T"] = np.ascontiguousarray((xT * xT).reshape(K1, 128, bl).astype(f8))
        m["e1"] = eT(eps1, b0)
        m["e2"] = eT(eps2, b0)
        m["e3"] = eT(eps3, b0)
        m["e4"] = e4T(eps4, b0)
        in_maps.append(m)
    return in_maps


def gather_output_fast(results, bl=BL, n_pairs=S // 2):
    ns = 2 * n_pairs
    out = np.empty((ns, N_CORES * bl, C), dtype=np.float32)
    for i, r in enumerate(results):
        oc = np.asarray(r["out"])  # [n_pairs, C, fd]
        oc = oc.reshape(n_pairs, C, 2, bl).transpose(0, 2, 3, 1).reshape(ns, bl, C)
        out[:, i * bl:(i + 1) * bl, :] = oc
    return out


def _uniform_scales(inputs):
    """Return (c1, c2, c3, c4) if every scale array is a constant fill."""
    cs = []
    for nm, drop in (("a1_scale", "a1_dropout"), ("a2_scale", "a2_dropout"),
                     ("a3_scale", "a3_dropout"), ("a4_scale", None)):
        s = np.asarray(inputs[nm], np.float32)
        if s.size == 0 or float(s.max()) != float(s.min()):
            return None
        d = float(np.asarray(inputs[drop], np.float32)) if drop else 1.0
        v = d * float(s.flat[0])
        cs.append(v * v)
    return tuple(cs)


# --------------------------------------------------------------------------
# General fallback path (baseline program, arbitrary scale arrays)
# --------------------------------------------------------------------------

def build_program_general(bl=BL, n_pairs=S // 2, act_lrelu=True):
    """Build the per-core Bass program. All cores run the same program (SPMD)."""
    fd = 2 * bl              # free dim per sample-pair
    nc = bacc.Bacc("TRN2", target_bir_lowering=False, debug=False)

    # ---- DRAM I/O (per core) ----
    xT_d = nc.dram_tensor("xT", [K1, 128, bl], F32R, kind="ExternalInput")
    x2T_d = nc.dram_tensor("x2T", [K1, 128, bl], F8E4, kind="ExternalInput")
    w1m_d = nc.dram_tensor("w1m", [K1, 128, H], F32R, kind="ExternalInput")
    w1v_d = nc.dram_tensor("w1v", [K1, 128, H], BF16, kind="ExternalInput")
    w2m_d = nc.dram_tensor("w2m", [KH, 128, H], F32R, kind="ExternalInput")
    w2v_d = nc.dram_tensor("w2v", [KH, 128, H], BF16, kind="ExternalInput")
    w3m_d = nc.dram_tensor("w3m", [KH, 128, H], F32R, kind="ExternalInput")
    w3v_d = nc.dram_tensor("w3v", [KH, 128, H], BF16, kind="ExternalInput")
    w4m_d = nc.dram_tensor("w4m", [128, KH * C], F32R, kind="ExternalInput")
    w4v_d = nc.dram_tensor("w4v", [128, KH * C], BF16, kind="ExternalInput")
    b2m_d = nc.dram_tensor("b2m", [1, H], F32R, kind="ExternalInput")
    b3m_d = nc.dram_tensor("b3m", [1, H], F32R, kind="ExternalInput")
    b4m_d = nc.dram_tensor("b4m", [1, C], F32R, kind="ExternalInput")
    b2v_d = nc.dram_tensor("b2v", [128, FO], F32, kind="ExternalInput")
    b3v_d = nc.dram_tensor("b3v", [128, FO], F32, kind="ExternalInput")
    b4v_d = nc.dram_tensor("b4v", [C, 1], F32, kind="ExternalInput")
    e1_d = nc.dram_tensor("e1", [n_pairs, FO, 128, fd], BF16, kind="ExternalInput")
    e2_d = nc.dram_tensor("e2", [n_pairs, FO, 128, fd], BF16, kind="ExternalInput")
    e3_d = nc.dram_tensor("e3", [n_pairs, FO, 128, fd], BF16, kind="ExternalInput")
    e4_d = nc.dram_tensor("e4", [n_pairs, C, fd], F32, kind="ExternalInput")
    b2mP_d = nc.dram_tensor("b2mP", [128, FO], F32, kind="ExternalInput")
    b3mP_d = nc.dram_tensor("b3mP", [128, FO], F32, kind="ExternalInput")
    ones_row_d = nc.dram_tensor("ones_row_in", [1, fd], F32R, kind="ExternalInput")
    ones10_d = nc.dram_tensor("ones10_in", [C, 1], F32R, kind="ExternalInput")
    out_d = nc.dram_tensor("out", [C, n_pairs * fd], F32, kind="ExternalOutput")

    def mm(out_ap, lhsT_ap, rhs_ap, start, stop):
        nc.tensor.matmul(out_ap, lhsT_ap, rhs_ap, start=start, stop=stop)

    with tile.TileContext(nc) as tc:
        with (
            tc.tile_pool(name="wp", bufs=1) as wp,        # persistent weights
            tc.tile_pool(name="sp", bufs=1) as sp,        # persistent activations
        ):
            # persistent weight tiles
            w2m_t = [wp.tile([128, H], F32R, tag=f"w2m{k}", name=f"w2m{k}") for k in range(KH)]
            w2v_t = [wp.tile([128, H], BF16, tag=f"w2v{k}", name=f"w2v{k}") for k in range(KH)]
            w3m_t = [wp.tile([128, H], F32R, tag=f"w3m{k}", name=f"w3m{k}") for k in range(KH)]
            w3v_t = [wp.tile([128, H], BF16, tag=f"w3v{k}", name=f"w3v{k}") for k in range(KH)]
            w4m_t = wp.tile([128, KH * C], F32R, tag="w4m", name="w4m")
            w4v_t = wp.tile([128, KH * C], BF16, tag="w4v", name="w4v")
            b2m_t = wp.tile([1, H], F32R, tag="b2m", name="b2m")
            b3m_t = wp.tile([1, H], F32R, tag="b3m", name="b3m")
            b4m_t = wp.tile([1, C], F32R, tag="b4m", name="b4m")
            b2v_t = wp.tile([128, FO], F32, tag="b2v", name="b2v")
            b3v_t = wp.tile([128, FO], F32, tag="b3v", name="b3v")
            b4v_t = wp.tile([C, 1], F32, tag="b4v", name="b4v")
            ones_row = wp.tile([1, fd], F32R, tag="ones_row", name="ones_row")
            ones10 = wp.tile([C, 1], F32R, tag="ones10", name="ones10")
            b2mP_t = wp.tile([128, FO], F32, tag="b2mP", name="b2mP")
            b3mP_t = wp.tile([128, FO], F32, tag="b3mP", name="b3mP")
            eps12_t = wp.tile([128, 1], F32, tag="eps12", name="eps12")
            z128_t = wp.tile([128, 1], F32, tag="z128", name="z128")
            zC_t = wp.tile([C, 1], F32, tag="zC", name="zC")
            z1_t = wp.tile([1, 1], F32, tag="z1", name="z1")
            nc.vector.memset(eps12_t[:], 1e-12)
            nc.vector.memset(z128_t[:], 0.0)
            nc.vector.memset(zC_t[:], 0.0)
            nc.vector.memset(z1_t[:], 0.0)

            # persistent per-core activations: mu1/sig1 (shared by all samples)
            mu1_t = sp.tile([128, FO * bl], F32, tag="mu1", name="mu1")
            sig1_t = sp.tile([128, FO * bl], F32, tag="sig1", name="sig1")
            u4_all = sp.tile([C, n_pairs * fd], F32, tag="u4", name="u4")
            out_all = sp.tile([C, n_pairs * fd], F32, tag="outall", name="outall")

            # ---------- Phase A: layer-1 mean/std, sample-independent ----------
            with (
                tc.tile_pool(name="ap", bufs=1) as ap,
                tc.tile_pool(name="psA", bufs=4, space="PSUM") as psA,
            ):
                w1m_t = [ap.tile([128, H], F32R, tag=f"w1m{k}", name=f"w1m{k}") for k in range(K1)]
                w1v_t = [ap.tile([128, H], BF16, tag=f"w1v{k}", name=f"w1v{k}") for k in range(K1)]
                xT_t = [ap.tile([128, bl], F32R, tag=f"xT{k}", name=f"xT{k}") for k in range(K1)]
                x2T_t = [ap.tile([128, bl], BF16, tag=f"x2T{k}", name=f"x2T{k}") for k in range(K1)]
                for k in range(K1):
                    nc.sync.dma_start(w1m_t[k][:], w1m_d[k])
                    nc.sync.dma_start(w1v_t[k][:], w1v_d[k])
                    nc.sync.dma_start(xT_t[k][:], xT_d[k])
                    nc.sync.dma_start(x2T_t[k][:], x2T_d[k])
                for fo in range(FO):
                    mu_ps = psA.tile([128, bl], F32, tag="psA_mu", name="psA_mu")
                    var_ps = psA.tile([128, bl], F32, tag="psA_var", name="psA_var")
                    for k in range(K1):
                        mm(mu_ps[:], w1m_t[k][:, fo * 128:(fo + 1) * 128],
                           xT_t[k][:], start=(k == 0), stop=(k == K1 - 1))
                    for k in range(K1):
                        mm(var_ps[:], w1v_t[k][:, fo * 128:(fo + 1) * 128],
                           x2T_t[k][:], start=(k == 0), stop=(k == K1 - 1))
                    nc.scalar.copy(mu1_t[:, fo * bl:(fo + 1) * bl], mu_ps[:])
                    nc.scalar.activation(sig1_t[:, fo * bl:(fo + 1) * bl],
                                         var_ps[:], AF.Sqrt, bias=eps12_t[:])

            # ---------- Phase B: per sample-pair, layers 1-4 ----------
            with (
                tc.tile_pool(name="ep", bufs=3) as ep,
                tc.tile_pool(name="hp", bufs=2) as hp,
                tc.tile_pool(name="tp", bufs=10) as tp,
                tc.tile_pool(name="psB", bufs=3, space="PSUM") as psB,
                tc.tile_pool(name="ps4", bufs=1, space="PSUM") as ps4,
            ):
                def emit_L1(p):
                    e1_t = [ep.tile([128, fd], BF16, tag=f"e1_{k}", name=f"e1_{k}")
                            for k in range(FO)]
                    for k in range(FO):
                        nc.sync.dma_start(e1_t[k][:], e1_d[p, k])
                    h1_t, h1q_t = [], []
                    for fo in range(FO):
                        sig_b = (sig1_t[:, fo * bl:(fo + 1) * bl]
                                 .unsqueeze(1).broadcast_to((128, 2, bl)))
                        mu_b = (mu1_t[:, fo * bl:(fo + 1) * bl]
                                .unsqueeze(1).broadcast_to((128, 2, bl)))
                        t_t = tp.tile([128, fd], F32, tag="tmp", name="tmp")
                        nc.vector.tensor_tensor(
                            t_t[:].rearrange("p (s n) -> p s n", s=2),
                            e1_t[fo][:].rearrange("p (s n) -> p s n", s=2),
                            sig_b, ALU.mult)
                        u_t = tp.tile([128, fd], F32, tag="tmp", name="tmp")
                        nc.vector.tensor_tensor(
                            u_t[:].rearrange("p (s n) -> p s n", s=2),
                            t_t[:].rearrange("p (s n) -> p s n", s=2),
                            mu_b, ALU.add)
                        h = hp.tile([128, fd], F32R, tag=f"h1_{fo}", name=f"h1_{fo}")
                        nc.scalar.activation(h[:], u_t[:], AF.Prelu,
                                             bias=z128_t[:], alpha=0.01)
                        hq = hp.tile([128, fd], BF16, tag=f"h1q_{fo}", name=f"h1q_{fo}")
                        nc.gpsimd.tensor_mul(hq[:], h[:], h[:])
                        h1_t.append(h)
                        h1q_t.append(hq)
                    return h1_t, h1q_t

                def hidden_layer(p, e_d, hin, hinq, wm_t, wv_t, bm_t, bmP_t, bv_t,
                                 htag):
                    eps_t = [ep.tile([128, fd], BF16, tag=f"{htag}e_{k}",
                                     name=f"{htag}e_{k}") for k in range(FO)]
                    for k in range(FO):
                        nc.sync.dma_start(eps_t[k][:], e_d[p, k])
                    hout, houtq = [], []
                    for fo in range(FO):
                        mu_ps = psB.tile([128, fd], F32, tag="psB_mu", name="psB_mu")
                        var_ps = psB.tile([128, fd], F32, tag="psB_var", name="psB_var")
                        for k in range(KH):
                            mm(mu_ps[:], wm_t[k][:, fo * 128:(fo + 1) * 128],
                               hin[k][:], start=(k == 0), stop=(k == KH - 1))
                        for k in range(KH):
                            mm(var_ps[:], wv_t[k][:, fo * 128:(fo + 1) * 128],
                               hinq[k][:], start=(k == 0), stop=(k == KH - 1))
                        sig_t = tp.tile([128, fd], F32, tag="tmp", name="tmp")
                        nc.scalar.activation(sig_t[:], var_ps[:], AF.Sqrt,
                                             bias=bv_t[:, fo:fo + 1])
                        t_t = tp.tile([128, fd], F32, tag="tmp", name="tmp")
                        nc.vector.tensor_tensor(t_t[:], sig_t[:], eps_t[fo][:],
                                                ALU.mult)
                        u_t = tp.tile([128, fd], F32, tag="tmp", name="tmp")
                        nc.vector.tensor_tensor(u_t[:], t_t[:], mu_ps[:], ALU.add)
                        h = hp.tile([128, fd], F32R, tag=f"{htag}_{fo}",
                                    name=f"{htag}_{fo}")
                        nc.scalar.activation(
                            h[:], u_t[:], AF.Prelu,
                            bias=bmP_t[:, fo:fo + 1], alpha=0.01)
                        hq = hp.tile([128, fd], BF16, tag=f"{htag}q_{fo}",
                                     name=f"{htag}q_{fo}")
                        nc.gpsimd.tensor_mul(hq[:], h[:], h[:])
                        hout.append(h)
                        houtq.append(hq)
                    return hout, houtq

                def emit_L4(p, h3_t, h3q_t):
                    e4_t = ep.tile([C, fd], F32, tag="e4", name="e4")
                    nc.sync.dma_start(e4_t[:], e4_d[p])
                    var4_ps = ps4.tile([C, fd], F32, tag="ps4_var", name="ps4_var")
                    for k in range(KH):
                        mm(var4_ps[:], w4v_t[:, k * C:(k + 1) * C], h3q_t[k][:],
                           start=(k == 0), stop=(k == KH - 1))
                    sig4_t = tp.tile([C, fd], F32, tag="tmp4", name="tmp4", bufs=4)
                    nc.scalar.activation(sig4_t[:], var4_ps[:], AF.Sqrt,
                                         bias=b4v_t[:])
                    t4_t = tp.tile([C, fd], F32, tag="tmp4", name="tmp4", bufs=4)
                    nc.vector.tensor_tensor(t4_t[:], sig4_t[:], e4_t[:], ALU.mult)
                    mu4_ps = ps4.tile([C, fd], F32, tag="ps4_mu", name="ps4_mu")
                    for k in range(KH):
                        mm(mu4_ps[:], w4m_t[:, k * C:(k + 1) * C], h3_t[k][:],
                           start=(k == 0), stop=False)
                    mm(mu4_ps[:], b4m_t[:], ones_row[:], start=False, stop=True)
                    nc.vector.tensor_tensor(u4_all[:, p * fd:(p + 1) * fd],
                                            t4_t[:], mu4_ps[:], ALU.add)

                # software pipeline: L1 of pair p+1 is emitted before the
                # heavy layers of pair p, so PE never idles between pairs
                h1_cur = emit_L1(0)
                for k in range(KH):
                    nc.sync.dma_start(w2m_t[k][:], w2m_d[k])
                    nc.sync.dma_start(w2v_t[k][:], w2v_d[k])
                nc.sync.dma_start(b2m_t[:], b2m_d[:])
                nc.sync.dma_start(b2v_t[:], b2v_d[:])
                nc.sync.dma_start(b2mP_t[:], b2mP_d[:])
                for k in range(KH):
                    nc.sync.dma_start(w3m_t[k][:], w3m_d[k])
                    nc.sync.dma_start(w3v_t[k][:], w3v_d[k])
                nc.sync.dma_start(b3m_t[:], b3m_d[:])
                nc.sync.dma_start(b3v_t[:], b3v_d[:])
                nc.sync.dma_start(b3mP_t[:], b3mP_d[:])
                nc.sync.dma_start(w4m_t[:], w4m_d[:])
                nc.sync.dma_start(w4v_t[:], w4v_d[:])
                nc.sync.dma_start(b4m_t[:], b4m_d[:])
                nc.sync.dma_start(b4v_t[:], b4v_d[:])
                nc.sync.dma_start(ones_row[:], ones_row_d[:])
                nc.sync.dma_start(ones10[:], ones10_d[:])
                for p in range(n_pairs):
                    h1_next = emit_L1(p + 1) if p + 1 < n_pairs else None
                    h1_t, h1q_t = h1_cur
                    h2_t, h2q_t = hidden_layer(p, e2_d, h1_t, h1q_t, w2m_t, w2v_t,
                                               b2m_t, b2mP_t, b2v_t, "h2")
                    h3_t, h3q_t = hidden_layer(p, e3_d, h2_t, h2q_t, w3m_t, w3v_t,
                                               b3m_t, b3mP_t, b3v_t, "h3")
                    emit_L4(p, h3_t, h3q_t)
                    h1_cur = h1_next

            # ---------- Phase C: log-softmax over C (exp/ln table) ----------
            with (
                tc.tile_pool(name="cp", bufs=2) as cp,
                tc.tile_pool(name="psC", bufs=2, space="PSUM") as psC,
            ):
                for p in range(n_pairs):
                    sl = slice(p * fd, (p + 1) * fd)
                    e_t = cp.tile([C, fd], F32R, tag="exp", name="exp")
                    nc.scalar.activation(e_t[:], u4_all[:, sl], AF.Exp, bias=zC_t[:])
                    s_ps = psC.tile([1, fd], F32, tag="psC_s", name="psC_s")
                    mm(s_ps[:], ones10[:], e_t[:], start=True, stop=True)
                    lse_t = cp.tile([1, fd], F32R, tag="lse", name="lse")
                    nc.scalar.activation(lse_t[:], s_ps[:], AF.Ln, bias=z1_t[:])
                    lseb_ps = psC.tile([C, fd], F32, tag="psC_b", name="psC_b")
                    mm(lseb_ps[:], ones_row[0:1, 0:C], lse_t[:], start=True, stop=True)
                    nc.vector.tensor_tensor(out_all[:, sl], u4_all[:, sl],
                                            lseb_ps[:], ALU.subtract)
                nc.sync.dma_start(out_d[:], out_all[:])

    nc.compile()
    return nc


def prepare_core_inputs_general(inputs, bl=BL, n_pairs=S // 2):
    """Host-side preprocessing: shard + transpose + fold parameters."""
    ns = 2 * n_pairs
    fd = 2 * bl
    f = np.float32
    x = np.asarray(inputs["inputs"], dtype=f)

    def padK(a):
        out = np.zeros((KPAD, a.shape[1]), dtype=f)
        out[:D_IN] = a
        return out

    w1m = padK(np.asarray(inputs["a1_mean"], f)).reshape(K1, 128, H)
    s1 = np.asarray(inputs["a1_dropout"], f) * np.asarray(inputs["a1_scale"], f)
    w1v = padK((s1 * s1).astype(f)).reshape(K1, 128, H).astype(bf)

    def hidden_w(mean, scale, dropout):
        m = np.asarray(mean, f)
        sc = (np.asarray(dropout, f) * np.asarray(scale, f)).astype(f)
        v = sc * sc
        wm = np.ascontiguousarray(m[:H].reshape(KH, 128, H))
        wv = np.ascontiguousarray(v[:H].reshape(KH, 128, H).astype(bf))
        bm = np.ascontiguousarray(m[H].reshape(1, H))
        bmP = np.ascontiguousarray(m[H].reshape(FO, 128).T)
        bv = np.ascontiguousarray((v[H] + np.float32(1e-12)).reshape(FO, 128).T)
        return wm, wv, bm, bmP, bv

    w2m, w2v, b2m, b2mP, b2v = hidden_w(inputs["a2_mean"], inputs["a2_scale"],
                                        inputs["a2_dropout"])
    w3m, w3v, b3m, b3mP, b3v = hidden_w(inputs["a3_mean"], inputs["a3_scale"],
                                        inputs["a3_dropout"])

    m4 = np.asarray(inputs["a4_mean"], f)
    s4 = np.asarray(inputs["a4_scale"], f)
    v4 = s4 * s4
    w4m = np.ascontiguousarray(m4[:H].reshape(KH, 128, C).transpose(1, 0, 2)
                               .reshape(128, KH * C))
    w4v = np.ascontiguousarray(v4[:H].reshape(KH, 128, C).transpose(1, 0, 2)
                               .reshape(128, KH * C).astype(bf))
    b4m = np.ascontiguousarray(m4[H].reshape(1, C))
    b4v = np.ascontiguousarray((v4[H] + np.float32(1e-12)).reshape(C, 1))

    shared = dict(w1m=w1m, w1v=w1v, w2m=w2m, w2v=w2v, w3m=w3m, w3v=w3v,
                  w4m=w4m, w4v=w4v, b2m=b2m, b3m=b3m, b4m=b4m,
                  b2mP=b2mP, b3mP=b3mP,
                  b2v=b2v, b3v=b3v, b4v=b4v,
                  ones_row_in=np.ones((1, fd), dtype=f),
                  ones10_in=np.ones((C, 1), dtype=f))

    eps1 = np.asarray(inputs["eps1"], f)
    eps2 = np.asarray(inputs["eps2"], f)
    eps3 = np.asarray(inputs["eps3"], f)
    eps4 = np.asarray(inputs["eps4"], f)

    def eT(e, b0):
        # [ns, bl, H] -> [n_pairs, FO(kchunk), 128, (si, b)]
        ec = e[:ns, b0:b0 + bl, :]
        return np.ascontiguousarray(
            ec.reshape(n_pairs, 2, bl, FO, 128).transpose(0, 3, 4, 1, 2)
            .reshape(n_pairs, FO, 128, fd).astype(ml_dtypes.bfloat16))

    def e4T(e, b0):
        ec = e[:ns, b0:b0 + bl, :]
        return np.ascontiguousarray(
            ec.reshape(n_pairs, 2, bl, C).transpose(0, 3, 1, 2)
            .reshape(n_pairs, C, fd))

    in_maps = []
    for i in range(N_CORES):
        b0 = i * bl
        xT = np.zeros((KPAD, bl), dtype=f)
        xT[:D_IN] = x[b0:b0 + bl].T
        x2T = (xT * xT).astype(ml_dtypes.bfloat16)
        m = dict(shared)
        m["xT"] = np.ascontiguousarray(xT.reshape(K1, 128, bl))
        m["x2T"] = np.ascontiguousarray(x2T.reshape(K1, 128, bl))
        m["e1"] = eT(eps1, b0)
        m["e2"] = eT(eps2, b0)
        m["e3"] = eT(eps3, b0)
        m["e4"] = e4T(eps4, b0)
        in_maps.append(m)
    return in_maps


def gather_output_general(results, bl=BL, n_pairs=S // 2):
    ns = 2 * n_pairs
    out = np.empty((ns, N_CORES * bl, C), dtype=np.float32)
    for i, r in enumerate(results):
        oc = np.asarray(r["out"])  # [C, n_pairs * fd]
        oc = oc.reshape(C, n_pairs, 2, bl).transpose(1, 2, 3, 0).reshape(ns, bl, C)
        out[:, i * bl:(i + 1) * bl, :] = oc
    return out


_CACHE = {}


def run(inputs, trace=False, **spmd_kwargs):
    cs = _uniform_scales(inputs)
    if cs is not None:
        key = ("fast",) + cs
        if key not in _CACHE:
            _CACHE[key] = build_program_fast(*cs)
        nc = _CACHE[key]
        in_maps = prepare_core_inputs_fast(inputs)
        res = run_bass_kernel_spmd(nc, in_maps, list(range(N_CORES)),
                                   trace=trace, **spmd_kwargs)
        return gather_output_fast(res.results), res
    key = ("general",)
    if key not in _CACHE:
        _CACHE[key] = build_program_general()
    nc = _CACHE[key]
    in_maps = prepare_core_inputs_general(inputs)
    res = run_bass_kernel_spmd(nc, in_maps, list(range(N_CORES)), trace=trace,
                               **spmd_kwargs)
    return gather_output_general(res.results), res


def kernel(**inputs):
    out, _ = run(inputs, trace=False)
    return out


# revision 24
# speedup vs baseline: 1.1712x; 1.1712x over previous
"""Trainium2 Bass kernel for a Bayesian MLP (local reparameterization trick).

Reference computation (per sample s of S=10):
    h1 = leaky_relu(x @ W1m + sqrt(x^2 @ W1v + 1e-12) * eps1_s)         [B, 512]
    h2 = leaky_relu(h1a @ W2m + sqrt(h1a^2 @ W2v + 1e-12) * eps2_s)     (h1a = [h1, 1])
    h3 = leaky_relu(h2a @ W3m + sqrt(h2a^2 @ W3v + 1e-12) * eps3_s)
    out = log_softmax(h3a @ W4m + sqrt(h3a^2 @ W4v + 1e-12) * eps4_s)   [B, 10]

Distribution: data-parallel over the batch axis, B=2048 -> 8 cores x 256 rows.
Small variational parameters replicated on every core.

Fast path (used when every a*_scale array is a constant fill, which holds for
the reference setup where scale = 0.1 * ones):
    x^2 @ (c*ones) = c * ||x||^2  -> the whole variance path collapses to a
    rank-1 partition-sum matmul per layer + one sqrt on a [1, fd] row.
  * activations [feat on 128 partitions, (sample,batch) free], bf16 matmuls
  * sigma*eps is accumulated into the mean PSUM via an identity matmul, so a
    single ACT Prelu (with the folded mean-bias) reads PSUM once per tile
  * per-engine balance: DVE does sig*e products and h^2 squares, ACT does
    prelus/sqrts, Pool (gpsimd) does PSUM->SBUF sigma/u4 copies + L1 squares
  * log-softmax deferred to a final phase (exp/ln table loads once)

General path: the original baseline program (full variance matmuls, f32r).
"""

import sys
import os

for _p in ("/opt/trn_rl_repo",):
    if _p not in sys.path and os.path.isdir(_p):
        sys.path.insert(0, _p)

import numpy as np
import ml_dtypes

import concourse.bass as bass
import concourse.bacc as bacc
import concourse.mybir as mybir
from concourse import tile
from concourse.bass_utils import run_bass_kernel_spmd

F32 = mybir.dt.float32
F32R = mybir.dt.float32r
BF16 = mybir.dt.bfloat16
F8E4 = mybir.dt.float8e4
AF = mybir.ActivationFunctionType
ALU = mybir.AluOpType

B, D_IN, H, C, S = 2048, 784, 512, 10, 10
N_CORES = 8
BL = B // N_CORES            # 256 rows per core
KPAD = 896                   # 784 padded to 7*128
K1 = KPAD // 128             # 7 k-chunks for layer 1
KH = H // 128                # 4 k-chunks for hidden layers
FO = H // 128                # 4 output-feature chunks for hidden layers

bf = ml_dtypes.bfloat16


# --------------------------------------------------------------------------
# Fast path: uniform scale arrays -> rank-1 variance
# --------------------------------------------------------------------------

def build_program_fast(c1, c2, c3, c4, bl=BL, n_pairs=S // 2):
    """Per-core program exploiting sigma_l^2 = c_l * (||h||^2 [+ 1]).

    sqrt is computed as exp(0.5*ln(.)) so the whole kernel runs inside the
    single natural_log_exp ACT table set; log-softmax interleaves per pair.
    sigma rows broadcast across partitions via gpsimd partition_broadcast.
    """
    fd = 2 * bl              # 512 free dim per sample-pair
    nc = bacc.Bacc("TRN2", target_bir_lowering=False, debug=False)

    # ---- DRAM I/O (per core) ----
    xT_d = nc.dram_tensor("xT", [K1, 128, bl], F8E4, kind="ExternalInput")
    x2T_d = nc.dram_tensor("x2T", [K1, 128, bl], F8E4, kind="ExternalInput")
    w1m_d = nc.dram_tensor("w1m", [K1, 128, H], F8E4, kind="ExternalInput")
    w2m_d = nc.dram_tensor("w2m", [KH, 128, H], BF16, kind="ExternalInput")
    w3m_d = nc.dram_tensor("w3m", [KH, 128, H], BF16, kind="ExternalInput")
    w4m_d = nc.dram_tensor("w4m", [128, KH * C], BF16, kind="ExternalInput")
    b2mP_d = nc.dram_tensor("b2mP", [128, FO], F32, kind="ExternalInput")
    b3mP_d = nc.dram_tensor("b3mP", [128, FO], F32, kind="ExternalInput")
    b4m_d = nc.dram_tensor("b4m", [1, C], BF16, kind="ExternalInput")
    e1_d = nc.dram_tensor("e1", [n_pairs, FO, 128, fd], BF16, kind="ExternalInput")
    e2_d = nc.dram_tensor("e2", [n_pairs, FO, 128, fd], BF16, kind="ExternalInput")
    e3_d = nc.dram_tensor("e3", [n_pairs, FO, 128, fd], BF16, kind="ExternalInput")
    e4_d = nc.dram_tensor("e4", [n_pairs, C, fd], BF16, kind="ExternalInput")
    eye128_d = nc.dram_tensor("eye128", [128, 128], BF16, kind="ExternalInput")
    eye10_d = nc.dram_tensor("eye10", [C, C], BF16, kind="ExternalInput")
    o_1x10_d = nc.dram_tensor("o_1x10", [1, C], BF16, kind="ExternalInput")
    n_1x10_d = nc.dram_tensor("n_1x10", [1, C], BF16, kind="ExternalInput")
    o_sq_d = nc.dram_tensor("o_sq", [128, 128], BF16, kind="ExternalInput")
    o_128x10_d = nc.dram_tensor("o_128x10", [128, C], BF16, kind="ExternalInput")
    o_128x1_d = nc.dram_tensor("o_128x1", [128, 1], BF16, kind="ExternalInput")
    o_10x1_d = nc.dram_tensor("o_10x1", [C, 1], BF16, kind="ExternalInput")
    ones_row_d = nc.dram_tensor("ones_row", [1, fd], BF16, kind="ExternalInput")
    out_d = nc.dram_tensor("out", [n_pairs, C, fd], F32, kind="ExternalOutput")

    mm = nc.tensor.matmul

    with tile.TileContext(nc) as tc:
        with (
            tc.tile_pool(name="wp", bufs=1) as wp,
        ):
            # persistent tiles
            w2m_t = [wp.tile([128, H], BF16, tag=f"w2m{k}", name=f"w2m{k}") for k in range(KH)]
            w3m_t = [wp.tile([128, H], BF16, tag=f"w3m{k}", name=f"w3m{k}") for k in range(KH)]
            w4m_t = wp.tile([128, KH * C], BF16, tag="w4m", name="w4m")
            b2mP_t = wp.tile([128, FO], F32, tag="b2mP", name="b2mP")
            b3mP_t = wp.tile([128, FO], F32, tag="b3mP", name="b3mP")
            b4m_t = wp.tile([1, C], BF16, tag="b4m", name="b4m")
            eye128_t = wp.tile([128, 128], BF16, tag="eye128", name="eye128")
            eye10_t = wp.tile([C, C], BF16, tag="eye10", name="eye10")
            o_1x10 = wp.tile([1, C], BF16, tag="o_1x10", name="o_1x10")
            n_1x10 = wp.tile([1, C], BF16, tag="n_1x10", name="n_1x10")
            o_sq = wp.tile([128, 128], BF16, tag="o_sq", name="o_sq")
            o_128x10 = wp.tile([128, C], BF16, tag="o_128x10", name="o_128x10")
            o_128x1 = wp.tile([128, 1], BF16, tag="o_128x1", name="o_128x1")
            o_10x1 = wp.tile([C, 1], BF16, tag="o_10x1", name="o_10x1")
            ones_row = wp.tile([1, fd], BF16, tag="ones_row", name="ones_row")
            mu1_t = wp.tile([128, FO * bl], BF16, tag="mu1", name="mu1")
            sig1b_t = wp.tile([128, bl], BF16, tag="sig1b", name="sig1b")
            z128_t = wp.tile([128, 1], F32, tag="z128", name="z128")
            zC_t = wp.tile([C, 1], F32, tag="zC", name="zC")
            z1_t = wp.tile([1, 1], F32, tag="z1", name="z1")
            cb_t = {}
            for nm, cv, npart in (("c2", c2, 128), ("c3", c3, 128),
                                  ("c4", c4, C)):
                cb_t[nm] = wp.tile([npart, 1], F32, tag=f"cb_{nm}",
                                   name=f"cb_{nm}")
                nc.vector.memset(cb_t[nm][:], float(cv))
            nc.vector.memset(z128_t[:], 0.0)
            nc.vector.memset(zC_t[:], 0.0)
            nc.vector.memset(z1_t[:], 0.0)

            # phase-A-critical small constants first
            nc.sync.dma_start(o_128x1[:], o_128x1_d[:])
            nc.sync.dma_start(o_sq[:], o_sq_d[:])

            # ---------- Phase A: layer-1 mu / sigma, sample-independent ----------
            with (
                tc.tile_pool(name="ap", bufs=1) as ap,
                tc.tile_pool(name="psA", bufs=1, space="PSUM") as psA,
            ):
                w1m_t = [ap.tile([128, H], F8E4, tag=f"w1m{k}", name=f"w1m{k}") for k in range(K1)]
                xT_t = [ap.tile([128, bl], F8E4, tag=f"xT{k}", name=f"xT{k}") for k in range(K1)]
                x2T_t = [ap.tile([128, bl], F8E4, tag=f"x2T{k}", name=f"x2T{k}") for k in range(K1)]
                o_sq8 = ap.tile([128, 128], F8E4, tag="o_sq8", name="o_sq8")
                nc.vector.memset(o_sq8[:], 1.0)
                # ~3.5us of dummy matmuls on memset tiles: wakes the PE HAM
                # clock gate (idle default is 1.2 GHz; sustained activity
                # unlocks 2.4 GHz) before the DMA-dependent real work lands.
                warm_rhs = ap.tile([128, fd], F8E4, tag="warm", name="warm")
                nc.vector.memset(warm_rhs[:], 0.0)
                warm_ps = psA.tile([128, fd], F32, tag="psA_w", name="psA_w")
                for i in range(9):
                    mm(warm_ps[:], o_sq8[:], warm_rhs[:], start=True, stop=True)
                for k in range(K1):
                    nc.sync.dma_start(x2T_t[k][:], x2T_d[k])
                for k in range(K1):
                    nc.sync.dma_start(xT_t[k][:], xT_d[k])
                    nc.sync.dma_start(w1m_t[k][:], w1m_d[k])

                # sigma1 first (it gates pair-0's L1 chain), then mu fo-major
                # so mu1[fo0] lands early and L1(0) overlaps the rest
                mu_ps = [psA.tile([128, bl], F32, tag=f"psA_mu{fo}",
                                  name=f"psA_mu{fo}") for fo in range(FO)]
                s1_ps = psA.tile([128, bl], F32, tag="psA_s1", name="psA_s1")
                for k in range(K1):
                    mm(s1_ps[:], o_sq8[:], x2T_t[k][:],
                       start=(k == 0), stop=(k == K1 - 1))
                lnv1 = ap.tile([128, bl], F32, tag="lnv1", name="lnv1")
                nc.scalar.activation(lnv1[:], s1_ps[:], AF.Ln, scale=float(c1),
                                     bias=z128_t[:])
                nc.scalar.activation(sig1b_t[:], lnv1[:], AF.Exp, scale=0.5,
                                     bias=z128_t[:])
                for fo in range(FO):
                    for k in range(K1):
                        mm(mu_ps[fo][:], w1m_t[k][:, fo * 128:(fo + 1) * 128],
                           xT_t[k][:], start=(k == 0), stop=(k == K1 - 1))
                    nc.vector.tensor_scalar_mul(
                        mu1_t[:, fo * bl:(fo + 1) * bl], mu_ps[fo][:],
                        1.0 / 16.0)

            # ---------- Phase B: per sample-pair, layers 1-4 + softmax ----------
            with (
                tc.tile_pool(name="ep", bufs=3) as ep,
                tc.tile_pool(name="hp", bufs=2) as hp,
                tc.tile_pool(name="tp", bufs=8) as tp,
                tc.tile_pool(name="sgp", bufs=2) as sgp,
                tc.tile_pool(name="psU", bufs=1, space="PSUM") as psU,
                tc.tile_pool(name="psV", bufs=1, space="PSUM") as psV,
            ):
                def dma_eps(p, with_weights=None):
                    e_t = {}
                    for nm, e_d in (("e1", e1_d), ("e2", e2_d), ("e3", e3_d)):
                        e_t[nm] = [ep.tile([128, fd], BF16, tag=f"{nm}_{k}",
                                           name=f"{nm}_{k}") for k in range(FO)]
                        for k in range(FO):
                            nc.sync.dma_start(e_t[nm][k][:], e_d[p, k])
                    e_t["e4"] = ep.tile([C, fd], BF16, tag="e4", name="e4")
                    nc.sync.dma_start(e_t["e4"][:], e4_d[p])
                    return e_t

                def emit_L1(eps):
                    """u1 = mu1 + sig1*e1 entirely in SBUF (no PE)."""
                    h_t, hq_t = [], []
                    for fo in range(FO):
                        sl = slice(fo * bl, (fo + 1) * bl)
                        sig_b = (sig1b_t[:].unsqueeze(1)
                                 .broadcast_to((128, 2, bl)))
                        mu_b = (mu1_t[:, sl].unsqueeze(1)
                                .broadcast_to((128, 2, bl)))
                        t_t = tp.tile([128, fd], BF16, tag="t1", name="t1", bufs=4)
                        nc.vector.tensor_tensor(
                            t_t[:].rearrange("p (s n) -> p s n", s=2),
                            eps["e1"][fo][:].rearrange("p (s n) -> p s n", s=2),
                            sig_b, ALU.mult)
                        u_t = tp.tile([128, fd], BF16, tag="u1", name="u1", bufs=4)
                        nc.vector.tensor_tensor(
                            u_t[:].rearrange("p (s n) -> p s n", s=2),
                            t_t[:].rearrange("p (s n) -> p s n", s=2),
                            mu_b, ALU.add)
                        h = hp.tile([128, fd], BF16, tag=f"h1_{fo}", name=f"h1_{fo}")
                        nc.scalar.activation(h[:], u_t[:], AF.Prelu,
                                             bias=z128_t[:], alpha=0.01)
                        hq = hp.tile([128, fd], BF16, tag=f"h1q_{fo}",
                                     name=f"h1q_{fo}")
                        if fo % 2 == 0:
                            nc.vector.tensor_tensor(hq[:], h[:], h[:], ALU.mult)
                        else:
                            nc.gpsimd.tensor_mul(hq[:], h[:], h[:])
                        h_t.append(h)
                        hq_t.append(hq)
                    return h_t, hq_t

                def emit_varones(hq_t, lhs, npart):
                    """||h||^2 summed over partitions AND broadcast to npart
                    partitions in one go (all-ones stationary matrix)."""
                    v_ps = psV.tile([128, fd], F32, tag="var", name="var", bufs=2)
                    for k in range(KH):
                        mm(v_ps[0:npart, :], lhs[:, 0:npart], hq_t[k][:],
                           start=(k == 0), stop=(k == KH - 1))
                    return v_ps

                def sigma_tail(v_ps, c, cb, npart, tag):
                    """sigma = exp(0.5*ln(c*S + c)) on the broadcast tile."""
                    lnv = tp.tile([npart, fd], F32, tag=f"lnv{tag}",
                                  name=f"lnv{tag}", bufs=2)
                    nc.scalar.activation(lnv[:], v_ps[0:npart, :], AF.Ln,
                                         scale=float(c), bias=cb[:])
                    sigb = sgp.tile([npart, fd], BF16, tag=f"sigb{tag}",
                                    name=f"sigb{tag}")
                    zb = z128_t if npart == 128 else zC_t
                    nc.scalar.activation(sigb[:], lnv[:], AF.Exp,
                                         scale=0.5, bias=zb[:])
                    return sigb

                def emit_hidden(eps_l, hin, sigb, wm_t, bmP_t, htag, vnext):
                    """One hidden layer; accumulates next layer's ||h||^2 into
                    vnext as each hq chunk completes (keeps the sigma chain of
                    the NEXT layer off the PE critical path)."""
                    hout, houtq = [], []
                    for fo in range(FO):
                        u_ps = psU.tile([128, fd], F32, tag=f"u{fo}", name=f"u{fo}")
                        for k in range(KH):
                            mm(u_ps[:], wm_t[k][:, fo * 128:(fo + 1) * 128],
                               hin[k][:], start=(k == 0), stop=False)
                        t_t = tp.tile([128, fd], BF16, tag="t", name="t", bufs=4)
                        nc.vector.tensor_tensor(t_t[:], eps_l[fo][:], sigb[:],
                                                ALU.mult)
                        mm(u_ps[:], eye128_t[:], t_t[:], start=False, stop=True)
                        h = hp.tile([128, fd], BF16, tag=f"{htag}_{fo}",
                                    name=f"{htag}_{fo}")
                        nc.scalar.activation(h[:], u_ps[:], AF.Prelu,
                                             bias=bmP_t[:, fo:fo + 1], alpha=0.01)
                        hq = hp.tile([128, fd], BF16, tag=f"{htag}q_{fo}",
                                     name=f"{htag}q_{fo}")
                        nc.vector.tensor_tensor(hq[:], h[:], h[:], ALU.mult)
                        if vnext is not None:
                            vn, vlhs, vnp = vnext
                            mm(vn[0:vnp, :], vlhs[:, 0:vnp], hq[:],
                               start=(fo == 0), stop=(fo == FO - 1))
                        hout.append(h)
                        houtq.append(hq)
                    return hout, houtq

                def emit_L4(p, h3_t, sigb4, eps):
                    t4 = tp.tile([C, fd], BF16, tag="t4", name="t4", bufs=2)
                    nc.vector.tensor_tensor(t4[:], eps["e4"][:], sigb4[:], ALU.mult)
                    u4_ps = psU.tile([C, fd], F32, tag="u4p", name="u4p")
                    for k in range(KH):
                        mm(u4_ps[:], w4m_t[:, k * C:(k + 1) * C], h3_t[k][:],
                           start=(k == 0), stop=False)
                    mm(u4_ps[:], b4m_t[:], ones_row[:], start=False, stop=False)
                    mm(u4_ps[:], eye10_t[:], t4[:], start=False, stop=True)
                    return u4_ps

                def emit_softmax(p, u4_ps):
                    # log-softmax in place on the u4 PSUM bank
                    e_t = tp.tile([C, fd], BF16, tag="expt", name="expt", bufs=2)
                    nc.scalar.activation(e_t[:], u4_ps[:], AF.Exp, bias=zC_t[:])
                    s_ps = psV.tile([128, fd], F32, tag="var", name="var", bufs=2)
                    mm(s_ps[0:1, :], o_10x1[:], e_t[:], start=True, stop=True)
                    lse_t = tp.tile([1, fd], BF16, tag="lse", name="lse", bufs=2)
                    nc.scalar.activation(lse_t[:], s_ps[0:1, :], AF.Ln,
                                         bias=z1_t[:])
                    mm(u4_ps[:], n_1x10[:], lse_t[:], start=False, stop=True,
                       skip_group_check=True)
                    o_t = tp.tile([C, fd], F32, tag="oct", name="oct", bufs=2)
                    nc.vector.tensor_copy(o_t[:], u4_ps[:])
                    nc.sync.dma_start(out_d[p], o_t[:])

                # prologue: eps(0) first in the DMA queue, then weights, eps(1)
                eps_cur = dma_eps(0)
                for k in range(KH):
                    nc.sync.dma_start(w2m_t[k][:], w2m_d[k])
                nc.sync.dma_start(b2mP_t[:], b2mP_d[:])
                nc.sync.dma_start(eye128_t[:], eye128_d[:])
                eps_next = dma_eps(1) if n_pairs > 1 else None
                for k in range(KH):
                    nc.sync.dma_start(w3m_t[k][:], w3m_d[k])
                nc.sync.dma_start(b3mP_t[:], b3mP_d[:])
                nc.sync.dma_start(w4m_t[:], w4m_d[:])
                nc.sync.dma_start(b4m_t[:], b4m_d[:])
                nc.sync.dma_start(eye10_t[:], eye10_d[:])
                nc.sync.dma_start(o_1x10[:], o_1x10_d[:])
                nc.sync.dma_start(n_1x10[:], n_1x10_d[:])
                nc.sync.dma_start(o_10x1[:], o_10x1_d[:])
                nc.sync.dma_start(o_128x10[:], o_128x10_d[:])
                nc.sync.dma_start(ones_row[:], ones_row_d[:])

                h1_cur = emit_L1(eps_cur)
                sigb2_cur = sigma_tail(emit_varones(h1_cur[1], o_sq, 128), c2,
                                       cb_t["c2"], 128, "2")
                u4_prev = None
                for p in range(n_pairs):
                    h1_t, hq1_t = h1_cur
                    v3_ps = psV.tile([128, fd], F32, tag="var", name="var", bufs=2)
                    h2_t, hq2_t = emit_hidden(eps_cur["e2"], h1_t, sigb2_cur,
                                              w2m_t, b2mP_t, "h2",
                                              (v3_ps, o_sq, 128))
                    sigb3 = sigma_tail(v3_ps, c3, cb_t["c3"], 128, "3")
                    eps_pf = None
                    if p + 2 < n_pairs:
                        eps_pf = dma_eps(p + 2)
                    h1_next = emit_L1(eps_next) if eps_next is not None else None
                    if u4_prev is not None:
                        emit_softmax(p - 1, u4_prev)
                    v4_ps = psV.tile([128, fd], F32, tag="var", name="var", bufs=2)
                    h3_t, hq3_t = emit_hidden(eps_cur["e3"], h2_t, sigb3,
                                              w3m_t, b3mP_t, "h3",
                                              (v4_ps, o_128x10, C))
                    sigb4 = sigma_tail(v4_ps, c4, cb_t["c4"], C, "4")
                    if h1_next is not None:
                        sigb2_cur = sigma_tail(emit_varones(h1_next[1], o_sq,
                                                            128), c2,
                                               cb_t["c2"], 128, "2")
                    u4_prev = emit_L4(p, h3_t, sigb4, eps_cur)
                    h1_cur = h1_next
                    eps_cur, eps_next = eps_next, eps_pf
                emit_softmax(n_pairs - 1, u4_prev)

    import concourse.bacc as _bacc_mod
    _orig_gat = _bacc_mod.get_activation_tables

    def _pinned_tables(arch):
        tabs = _orig_gat(arch)
        keep = "natural_log_exp_and_others"
        return {nm: (fns if nm == keep else set()) for nm, fns in tabs.items()}

    _bacc_mod.get_activation_tables = _pinned_tables
    try:
        nc.compile()
    finally:
        _bacc_mod.get_activation_tables = _orig_gat
    return nc


def prepare_core_inputs_fast(inputs, bl=BL, n_pairs=S // 2):
    ns = 2 * n_pairs
    fd = 2 * bl
    f = np.float32
    x = np.asarray(inputs["inputs"], dtype=f)

    def padK(a):
        out = np.zeros((KPAD, a.shape[1]), dtype=f)
        out[:D_IN] = a
        return out

    f8 = ml_dtypes.float8_e4m3fn
    w1m = (padK(np.asarray(inputs["a1_mean"], f)) * np.float32(16.0)) \
        .reshape(K1, 128, H).astype(f8)

    def hidden_w(mean):
        m = np.asarray(mean, f)
        wm = np.ascontiguousarray(m[:H].reshape(KH, 128, H).astype(bf))
        bmP = np.ascontiguousarray(m[H].reshape(FO, 128).T.astype(f))
        return wm, bmP

    w2m, b2mP = hidden_w(inputs["a2_mean"])
    w3m, b3mP = hidden_w(inputs["a3_mean"])

    m4 = np.asarray(inputs["a4_mean"], f)
    w4m = np.ascontiguousarray(m4[:H].reshape(KH, 128, C).transpose(1, 0, 2)
                               .reshape(128, KH * C).astype(bf))
    b4m = np.ascontiguousarray(m4[H].reshape(1, C).astype(bf))

    shared = dict(
        w1m=w1m, w2m=w2m, w3m=w3m, w4m=w4m,
        b2mP=b2mP, b3mP=b3mP, b4m=b4m,
        eye128=np.eye(128, dtype=bf),
        eye10=np.eye(C, dtype=bf),
        o_1x10=np.ones((1, C), dtype=bf),
        o_128x1=np.ones((128, 1), dtype=bf),
        o_sq=np.ones((128, 128), dtype=bf),
        o_128x10=np.ones((128, C), dtype=bf),
        n_1x10=np.full((1, C), -1.0, dtype=bf),
        o_10x1=np.ones((C, 1), dtype=bf),
        ones_row=np.ones((1, fd), dtype=bf),
    )

    eps1 = np.asarray(inputs["eps1"], f)
    eps2 = np.asarray(inputs["eps2"], f)
    eps3 = np.asarray(inputs["eps3"], f)
    eps4 = np.asarray(inputs["eps4"], f)

    def eT(e, b0):
        ec = e[:ns, b0:b0 + bl, :]
        return np.ascontiguousarray(
            ec.reshape(n_pairs, 2, bl, FO, 128).transpose(0, 3, 4, 1, 2)
            .reshape(n_pairs, FO, 128, fd).astype(bf))

    def e4T(e, b0):
        ec = e[:ns, b0:b0 + bl, :]
        return np.ascontiguousarray(
            ec.reshape(n_pairs, 2, bl, C).transpose(0, 3, 1, 2)
            .reshape(n_pairs, C, fd).astype(bf))

    in_maps = []
    for i in range(N_CORES):
        b0 = i * bl
        xT = np.zeros((KPAD, bl), dtype=f)
        xT[:D_IN] = x[b0:b0 + bl].T
        f8 = ml_dtypes.float8_e4m3fn
        m = dict(shared)
        m["xT"] = np.ascontiguousarray(xT.reshape(K1, 128, bl).astype(f8))
        m["x2T"] = np.ascontiguousarray((xT * xT).reshape(K1, 128, bl).astype(f8))
        m["e1"] = eT(eps1, b0)
        m["e2"] = eT(eps2, b0)
        m["e3"] = eT(eps3, b0)
        m["e4"] = e4T(eps4, b0)
        in_maps.append(m)
    return in_maps


def gather_output_fast(results, bl=BL, n_pairs=S // 2):
    ns = 2 * n_pairs
    out = np.empty((ns, N_CORES * bl, C), dtype=np.float32)
    for i, r in enumerate(results):
        oc = np.asarray(r["out"])  # [n_pairs, C, fd]
        oc = oc.reshape(n_pairs, C, 2, bl).transpose(0, 2, 3, 1).reshape(ns, bl, C)
        out[:, i * bl:(i + 1) * bl, :] = oc
    return out


def _uniform_scales(inputs):
    """Return (c1, c2, c3, c4) if every scale array is a constant fill."""
    cs = []
    for nm, drop in (("a1_scale", "a1_dropout"), ("a2_scale", "a2_dropout"),
                     ("a3_scale", "a3_dropout"), ("a4_scale", None)):
        s = np.asarray(inputs[nm], np.float32)
        if s.size == 0 or float(s.max()) != float(s.min()):
            return None
        d = float(np.asarray(inputs[drop], np.float32)) if drop else 1.0
        v = d * float(s.flat[0])
        cs.append(v * v)
    return tuple(cs)


# --------------------------------------------------------------------------
# General fallback path (baseline program, arbitrary scale arrays)
# --------------------------------------------------------------------------

def build_program_general(bl=BL, n_pairs=S // 2, act_lrelu=True):
    """Build the per-core Bass program. All cores run the same program (SPMD)."""
    fd = 2 * bl              # free dim per sample-pair
    nc = bacc.Bacc("TRN2", target_bir_lowering=False, debug=False)

    # ---- DRAM I/O (per core) ----
    xT_d = nc.dram_tensor("xT", [K1, 128, bl], F32R, kind="ExternalInput")
    x2T_d = nc.dram_tensor("x2T", [K1, 128, bl], BF16, kind="ExternalInput")
    w1m_d = nc.dram_tensor("w1m", [K1, 128, H], F32R, kind="ExternalInput")
    w1v_d = nc.dram_tensor("w1v", [K1, 128, H], BF16, kind="ExternalInput")
    w2m_d = nc.dram_tensor("w2m", [KH, 128, H], F32R, kind="ExternalInput")
    w2v_d = nc.dram_tensor("w2v", [KH, 128, H], BF16, kind="ExternalInput")
    w3m_d = nc.dram_tensor("w3m", [KH, 128, H], F32R, kind="ExternalInput")
    w3v_d = nc.dram_tensor("w3v", [KH, 128, H], BF16, kind="ExternalInput")
    w4m_d = nc.dram_tensor("w4m", [128, KH * C], F32R, kind="ExternalInput")
    w4v_d = nc.dram_tensor("w4v", [128, KH * C], BF16, kind="ExternalInput")
    b2m_d = nc.dram_tensor("b2m", [1, H], F32R, kind="ExternalInput")
    b3m_d = nc.dram_tensor("b3m", [1, H], F32R, kind="ExternalInput")
    b4m_d = nc.dram_tensor("b4m", [1, C], F32R, kind="ExternalInput")
    b2v_d = nc.dram_tensor("b2v", [128, FO], F32, kind="ExternalInput")
    b3v_d = nc.dram_tensor("b3v", [128, FO], F32, kind="ExternalInput")
    b4v_d = nc.dram_tensor("b4v", [C, 1], F32, kind="ExternalInput")
    e1_d = nc.dram_tensor("e1", [n_pairs, FO, 128, fd], BF16, kind="ExternalInput")
    e2_d = nc.dram_tensor("e2", [n_pairs, FO, 128, fd], BF16, kind="ExternalInput")
    e3_d = nc.dram_tensor("e3", [n_pairs, FO, 128, fd], BF16, kind="ExternalInput")
    e4_d = nc.dram_tensor("e4", [n_pairs, C, fd], F32, kind="ExternalInput")
    b2mP_d = nc.dram_tensor("b2mP", [128, FO], F32, kind="ExternalInput")
    b3mP_d = nc.dram_tensor("b3mP", [128, FO], F32, kind="ExternalInput")
    ones_row_d = nc.dram_tensor("ones_row_in", [1, fd], F32R, kind="ExternalInput")
    ones10_d = nc.dram_tensor("ones10_in", [C, 1], F32R, kind="ExternalInput")
    out_d = nc.dram_tensor("out", [C, n_pairs * fd], F32, kind="ExternalOutput")

    def mm(out_ap, lhsT_ap, rhs_ap, start, stop):
        nc.tensor.matmul(out_ap, lhsT_ap, rhs_ap, start=start, stop=stop)

    with tile.TileContext(nc) as tc:
        with (
            tc.tile_pool(name="wp", bufs=1) as wp,        # persistent weights
            tc.tile_pool(name="sp", bufs=1) as sp,        # persistent activations
        ):
            # persistent weight tiles
            w2m_t = [wp.tile([128, H], F32R, tag=f"w2m{k}", name=f"w2m{k}") for k in range(KH)]
            w2v_t = [wp.tile([128, H], BF16, tag=f"w2v{k}", name=f"w2v{k}") for k in range(KH)]
            w3m_t = [wp.tile([128, H], F32R, tag=f"w3m{k}", name=f"w3m{k}") for k in range(KH)]
            w3v_t = [wp.tile([128, H], BF16, tag=f"w3v{k}", name=f"w3v{k}") for k in range(KH)]
            w4m_t = wp.tile([128, KH * C], F32R, tag="w4m", name="w4m")
            w4v_t = wp.tile([128, KH * C], BF16, tag="w4v", name="w4v")
            b2m_t = wp.tile([1, H], F32R, tag="b2m", name="b2m")
            b3m_t = wp.tile([1, H], F32R, tag="b3m", name="b3m")
            b4m_t = wp.tile([1, C], F32R, tag="b4m", name="b4m")
            b2v_t = wp.tile([128, FO], F32, tag="b2v", name="b2v")
            b3v_t = wp.tile([128, FO], F32, tag="b3v", name="b3v")
            b4v_t = wp.tile([C, 1], F32, tag="b4v", name="b4v")
            ones_row = wp.tile([1, fd], F32R, tag="ones_row", name="ones_row")
            ones10 = wp.tile([C, 1], F32R, tag="ones10", name="ones10")
            b2mP_t = wp.tile([128, FO], F32, tag="b2mP", name="b2mP")
            b3mP_t = wp.tile([128, FO], F32, tag="b3mP", name="b3mP")
            eps12_t = wp.tile([128, 1], F32, tag="eps12", name="eps12")
            z128_t = wp.tile([128, 1], F32, tag="z128", name="z128")
            zC_t = wp.tile([C, 1], F32, tag="zC", name="zC")
            z1_t = wp.tile([1, 1], F32, tag="z1", name="z1")
            nc.vector.memset(eps12_t[:], 1e-12)
            nc.vector.memset(z128_t[:], 0.0)
            nc.vector.memset(zC_t[:], 0.0)
            nc.vector.memset(z1_t[:], 0.0)

            # persistent per-core activations: mu1/sig1 (shared by all samples)
            mu1_t = sp.tile([128, FO * bl], F32, tag="mu1", name="mu1")
            sig1_t = sp.tile([128, FO * bl], F32, tag="sig1", name="sig1")
            u4_all = sp.tile([C, n_pairs * fd], F32, tag="u4", name="u4")
            out_all = sp.tile([C, n_pairs * fd], F32, tag="outall", name="outall")

            # ---------- Phase A: layer-1 mean/std, sample-independent ----------
            with (
                tc.tile_pool(name="ap", bufs=1) as ap,
                tc.tile_pool(name="psA", bufs=4, space="PSUM") as psA,
            ):
                w1m_t = [ap.tile([128, H], F32R, tag=f"w1m{k}", name=f"w1m{k}") for k in range(K1)]
                w1v_t = [ap.tile([128, H], BF16, tag=f"w1v{k}", name=f"w1v{k}") for k in range(K1)]
                xT_t = [ap.tile([128, bl], F32R, tag=f"xT{k}", name=f"xT{k}") for k in range(K1)]
                x2T_t = [ap.tile([128, bl], BF16, tag=f"x2T{k}", name=f"x2T{k}") for k in range(K1)]
                for k in range(K1):
                    nc.sync.dma_start(w1m_t[k][:], w1m_d[k])
                    nc.sync.dma_start(w1v_t[k][:], w1v_d[k])
                    nc.sync.dma_start(xT_t[k][:], xT_d[k])
                    nc.sync.dma_start(x2T_t[k][:], x2T_d[k])
                for fo in range(FO):
                    mu_ps = psA.tile([128, bl], F32, tag="psA_mu", name="psA_mu")
                    var_ps = psA.tile([128, bl], F32, tag="psA_var", name="psA_var")
                    for k in range(K1):
                        mm(mu_ps[:], w1m_t[k][:, fo * 128:(fo + 1) * 128],
                           xT_t[k][:], start=(k == 0), stop=(k == K1 - 1))
                    for k in range(K1):
                        mm(var_ps[:], w1v_t[k][:, fo * 128:(fo + 1) * 128],
                           x2T_t[k][:], start=(k == 0), stop=(k == K1 - 1))
                    nc.scalar.copy(mu1_t[:, fo * bl:(fo + 1) * bl], mu_ps[:])
                    nc.scalar.activation(sig1_t[:, fo * bl:(fo + 1) * bl],
                                         var_ps[:], AF.Sqrt, bias=eps12_t[:])

            # ---------- Phase B: per sample-pair, layers 1-4 ----------
            with (
                tc.tile_pool(name="ep", bufs=3) as ep,
                tc.tile_pool(name="hp", bufs=2) as hp,
                tc.tile_pool(name="tp", bufs=10) as tp,
                tc.tile_pool(name="psB", bufs=3, space="PSUM") as psB,
                tc.tile_pool(name="ps4", bufs=1, space="PSUM") as ps4,
            ):
                def emit_L1(p):
                    e1_t = [ep.tile([128, fd], BF16, tag=f"e1_{k}", name=f"e1_{k}")
                            for k in range(FO)]
                    for k in range(FO):
                        nc.sync.dma_start(e1_t[k][:], e1_d[p, k])
                    h1_t, h1q_t = [], []
                    for fo in range(FO):
                        sig_b = (sig1_t[:, fo * bl:(fo + 1) * bl]
                                 .unsqueeze(1).broadcast_to((128, 2, bl)))
                        mu_b = (mu1_t[:, fo * bl:(fo + 1) * bl]
                                .unsqueeze(1).broadcast_to((128, 2, bl)))
                        t_t = tp.tile([128, fd], F32, tag="tmp", name="tmp")
                        nc.vector.tensor_tensor(
                            t_t[:].rearrange("p (s n) -> p s n", s=2),
                            e1_t[fo][:].rearrange("p (s n) -> p s n", s=2),
                            sig_b, ALU.mult)
                        u_t = tp.tile([128, fd], F32, tag="tmp", name="tmp")
                        nc.vector.tensor_tensor(
                            u_t[:].rearrange("p (s n) -> p s n", s=2),
                            t_t[:].rearrange("p (s n) -> p s n", s=2),
                            mu_b, ALU.add)
                        h = hp.tile([128, fd], F32R, tag=f"h1_{fo}", name=f"h1_{fo}")
                        nc.scalar.activation(h[:], u_t[:], AF.Prelu,
                                             bias=z128_t[:], alpha=0.01)
                        hq = hp.tile([128, fd], BF16, tag=f"h1q_{fo}", name=f"h1q_{fo}")
                        nc.gpsimd.tensor_mul(hq[:], h[:], h[:])
                        h1_t.append(h)
                        h1q_t.append(hq)
                    return h1_t, h1q_t

                def hidden_layer(p, e_d, hin, hinq, wm_t, wv_t, bm_t, bmP_t, bv_t,
                                 htag):
                    eps_t = [ep.tile([128, fd], BF16, tag=f"{htag}e_{k}",
                                     name=f"{htag}e_{k}") for k in range(FO)]
                    for k in range(FO):
                        nc.sync.dma_start(eps_t[k][:], e_d[p, k])
                    hout, houtq = [], []
                    for fo in range(FO):
                        mu_ps = psB.tile([128, fd], F32, tag="psB_mu", name="psB_mu")
                        var_ps = psB.tile([128, fd], F32, tag="psB_var", name="psB_var")
                        for k in range(KH):
                            mm(mu_ps[:], wm_t[k][:, fo * 128:(fo + 1) * 128],
                               hin[k][:], start=(k == 0), stop=(k == KH - 1))
                        for k in range(KH):
                            mm(var_ps[:], wv_t[k][:, fo * 128:(fo + 1) * 128],
                               hinq[k][:], start=(k == 0), stop=(k == KH - 1))
                        sig_t = tp.tile([128, fd], F32, tag="tmp", name="tmp")
                        nc.scalar.activation(sig_t[:], var_ps[:], AF.Sqrt,
                                             bias=bv_t[:, fo:fo + 1])
                        t_t = tp.tile([128, fd], F32, tag="tmp", name="tmp")
                        nc.vector.tensor_tensor(t_t[:], sig_t[:], eps_t[fo][:],
                                                ALU.mult)
                        u_t = tp.tile([128, fd], F32, tag="tmp", name="tmp")
                        nc.vector.tensor_tensor(u_t[:], t_t[:], mu_ps[:], ALU.add)
                        h = hp.tile([128, fd], F32R, tag=f"{htag}_{fo}",
                                    name=f"{htag}_{fo}")
                        nc.scalar.activation(
                            h[:], u_t[:], AF.Prelu,
                            bias=bmP_t[:, fo:fo + 1], alpha=0.01)
                        hq = hp.tile([128, fd], BF16, tag=f"{htag}q_{fo}",
                                     name=f"{htag}q_{fo}")
                        nc.gpsimd.tensor_mul(hq[:], h[:], h[:])
                        hout.append(h)
                        houtq.append(hq)
                    return hout, houtq

                def emit_L4(p, h3_t, h3q_t):
                    e4_t = ep.tile([C, fd], F32, tag="e4", name="e4")
                    nc.sync.dma_start(e4_t[:], e4_d[p])
                    var4_ps = ps4.tile([C, fd], F32, tag="ps4_var", name="ps4_var")
                    for k in range(KH):
                        mm(var4_ps[:], w4v_t[:, k * C:(k + 1) * C], h3q_t[k][:],
                           start=(k == 0), stop=(k == KH - 1))
                    sig4_t = tp.tile([C, fd], F32, tag="tmp4", name="tmp4", bufs=4)
                    nc.scalar.activation(sig4_t[:], var4_ps[:], AF.Sqrt,
                                         bias=b4v_t[:])
                    t4_t = tp.tile([C, fd], F32, tag="tmp4", name="tmp4", bufs=4)
                    nc.vector.tensor_tensor(t4_t[:], sig4_t[:], e4_t[:], ALU.mult)
                    mu4_ps = ps4.tile([C, fd], F32, tag="ps4_mu", name="ps4_mu")
                    for k in range(KH):
                        mm(mu4_ps[:], w4m_t[:, k * C:(k + 1) * C], h3_t[k][:],
                           start=(k == 0), stop=False)
                    mm(mu4_ps[:], b4m_t[:], ones_row[:], start=False, stop=True)
                    nc.vector.tensor_tensor(u4_all[:, p * fd:(p + 1) * fd],
                                            t4_t[:], mu4_ps[:], ALU.add)

                # software pipeline: L1 of pair p+1 is emitted before the
                # heavy layers of pair p, so PE never idles between pairs
                h1_cur = emit_L1(0)
                for k in range(KH):
                    nc.sync.dma_start(w2m_t[k][:], w2m_d[k])
                    nc.sync.dma_start(w2v_t[k][:], w2v_d[k])
                nc.sync.dma_start(b2m_t[:], b2m_d[:])
                nc.sync.dma_start(b2v_t[:], b2v_d[:])
                nc.sync.dma_start(b2mP_t[:], b2mP_d[:])
                for k in range(KH):
                    nc.sync.dma_start(w3m_t[k][:], w3m_d[k])
                    nc.sync.dma_start(w3v_t[k][:], w3v_d[k])
                nc.sync.dma_start(b3m_t[:], b3m_d[:])
                nc.sync.dma_start(b3v_t[:], b3v_d[:])
                nc.sync.dma_start(b3mP_t[:], b3mP_d[:])
                nc.sync.dma_start(w4m_t[:], w4m_d[:])
                nc.sync.dma_start(w4v_t[:], w4v_d[:])
                nc.sync.dma_start(b4m_t[:], b4m_d[:])
                nc.sync.dma_start(b4v_t[:], b4v_d[:])
                nc.sync.dma_start(ones_row[:], ones_row_d[:])
                nc.sync.dma_start(ones10[:], ones10_d[:])
                for p in range(n_pairs):
                    h1_next = emit_L1(p + 1) if p + 1 < n_pairs else None
                    h1_t, h1q_t = h1_cur
                    h2_t, h2q_t = hidden_layer(p, e2_d, h1_t, h1q_t, w2m_t, w2v_t,
                                               b2m_t, b2mP_t, b2v_t, "h2")
                    h3_t, h3q_t = hidden_layer(p, e3_d, h2_t, h2q_t, w3m_t, w3v_t,
                                               b3m_t, b3mP_t, b3v_t, "h3")
                    emit_L4(p, h3_t, h3q_t)
                    h1_cur = h1_next

            # ---------- Phase C: log-softmax over C (exp/ln table) ----------
            with (
                tc.tile_pool(name="cp", bufs=2) as cp,
                tc.tile_pool(name="psC", bufs=2, space="PSUM") as psC,
            ):
                for p in range(n_pairs):
                    sl = slice(p * fd, (p + 1) * fd)
                    e_t = cp.tile([C, fd], F32R, tag="exp", name="exp")
                    nc.scalar.activation(e_t[:], u4_all[:, sl], AF.Exp, bias=zC_t[:])
                    s_ps = psC.tile([1, fd], F32, tag="psC_s", name="psC_s")
                    mm(s_ps[:], ones10[:], e_t[:], start=True, stop=True)
                    lse_t = cp.tile([1, fd], F32R, tag="lse", name="lse")
                    nc.scalar.activation(lse_t[:], s_ps[:], AF.Ln, bias=z1_t[:])
                    lseb_ps = psC.tile([C, fd], F32, tag="psC_b", name="psC_b")
                    mm(lseb_ps[:], ones_row[0:1, 0:C], lse_t[:], start=True, stop=True)
                    nc.vector.tensor_tensor(out_all[:, sl], u4_all[:, sl],
                                            lseb_ps[:], ALU.subtract)
                nc.sync.dma_start(out_d[:], out_all[:])

    nc.compile()
    return nc


def prepare_core_inputs_general(inputs, bl=BL, n_pairs=S // 2):
    """Host-side preprocessing: shard + transpose + fold parameters."""
    ns = 2 * n_pairs
    fd = 2 * bl
    f = np.float32
    x = np.asarray(inputs["inputs"], dtype=f)

    def padK(a):
        out = np.zeros((KPAD, a.shape[1]), dtype=f)
        out[:D_IN] = a
        return out

    w1m = padK(np.asarray(inputs["a1_mean"], f)).reshape(K1, 128, H)
    s1 = np.asarray(inputs["a1_dropout"], f) * np.asarray(inputs["a1_scale"], f)
    w1v = padK((s1 * s1).astype(f)).reshape(K1, 128, H).astype(bf)

    def hidden_w(mean, scale, dropout):
        m = np.asarray(mean, f)
        sc = (np.asarray(dropout, f) * np.asarray(scale, f)).astype(f)
        v = sc * sc
        wm = np.ascontiguousarray(m[:H].reshape(KH, 128, H))
        wv = np.ascontiguousarray(v[:H].reshape(KH, 128, H).astype(bf))
        bm = np.ascontiguousarray(m[H].reshape(1, H))
        bmP = np.ascontiguousarray(m[H].reshape(FO, 128).T)
        bv = np.ascontiguousarray((v[H] + np.float32(1e-12)).reshape(FO, 128).T)
        return wm, wv, bm, bmP, bv

    w2m, w2v, b2m, b2mP, b2v = hidden_w(inputs["a2_mean"], inputs["a2_scale"],
                                        inputs["a2_dropout"])
    w3m, w3v, b3m, b3mP, b3v = hidden_w(inputs["a3_mean"], inputs["a3_scale"],
                                        inputs["a3_dropout"])

    m4 = np.asarray(inputs["a4_mean"], f)
    s4 = np.asarray(inputs["a4_scale"], f)
    v4 = s4 * s4
    w4m = np.ascontiguousarray(m4[:H].reshape(KH, 128, C).transpose(1, 0, 2)
                               .reshape(128, KH * C))
    w4v = np.ascontiguousarray(v4[:H].reshape(KH, 128, C).transpose(1, 0, 2)
                               .reshape(128, KH * C).astype(bf))
    b4m = np.ascontiguousarray(m4[H].reshape(1, C))
    b4v = np.ascontiguousarray((v4[H] + np.float32(1e-12)).reshape(C, 1))

    shared = dict(w1m=w1m, w1v=w1v, w2m=w2m, w2v=w2v, w3m=w3m, w3v=w3v,
                  w4m=w4m, w4v=w4v, b2m=b2m, b3m=b3m, b4m=b4m,
                  b2mP=b2mP, b3mP=b3mP,
                  b2v=b2v, b3v=b3v, b4v=b4v,
                  ones_row_in=np.ones((1, fd), dtype=f),
                  ones10_in=np.ones((C, 1), dtype=f))

    eps1 = np.asarray(inputs["eps1"], f)
    eps2 = np.asarray(inputs["eps2"], f)
    eps3 = np.asarray(inputs["eps3"], f)
    eps4 = np.asarray(inputs["eps4"], f)

    def eT(e, b0):
        # [ns, bl, H] -> [n_pairs, FO(kchunk), 128, (si, b)]
        ec = e[:ns, b0:b0 + bl, :]
        return np.ascontiguousarray(
            ec.reshape(n_pairs, 2, bl, FO, 128).transpose(0, 3, 4, 1, 2)
            .reshape(n_pairs, FO, 128, fd).astype(ml_dtypes.bfloat16))

    def e4T(e, b0):
        ec = e[:ns, b0:b0 + bl, :]
        return np.ascontiguousarray(
            ec.reshape(n_pairs, 2, bl, C).transpose(0, 3, 1, 2)
            .reshape(n_pairs, C, fd))

    in_maps = []
    for i in range(N_CORES):
        b0 = i * bl
        xT = np.zeros((KPAD, bl), dtype=f)
        xT[:D_IN] = x[b0:b0 + bl].T
        x2T = (xT * xT).astype(ml_dtypes.bfloat16)
        m = dict(shared)
        m["xT"] = np.ascontiguousarray(xT.reshape(K1, 128, bl))
        m["x2T"] = np.ascontiguousarray(x2T.reshape(K1, 128, bl))
        m["e1"] = eT(eps1, b0)
        m["e2"] = eT(eps2, b0)
        m["e3"] = eT(eps3, b0)
        m["e4"] = e4T(eps4, b0)
        in_maps.append(m)
    return in_maps


def gather_output_general(results, bl=BL, n_pairs=S // 2):
    ns = 2 * n_pairs
    out = np.empty((ns, N_CORES * bl, C), dtype=np.float32)
    for i, r in enumerate(results):
        oc = np.asarray(r["out"])  # [C, n_pairs * fd]
        oc = oc.reshape(C, n_pairs, 2, bl).transpose(1, 2, 3, 0).reshape(ns, bl, C)
        out[:, i * bl:(i + 1) * bl, :] = oc
    return out


_CACHE = {}


def run(inputs, trace=False, **spmd_kwargs):
    cs = _uniform_scales(inputs)
    if cs is not None:
        key = ("fast",) + cs
        if key not in _CACHE:
            _CACHE[key] = build_program_fast(*cs)
        nc = _CACHE[key]
        in_maps = prepare_core_inputs_fast(inputs)
        res = run_bass_kernel_spmd(nc, in_maps, list(range(N_CORES)),
                                   trace=trace, **spmd_kwargs)
        return gather_output_fast(res.results), res
    key = ("general",)
    if key not in _CACHE:
        _CACHE[key] = build_program_general()
    nc = _CACHE[key]
    in_maps = prepare_core_inputs_general(inputs)
    res = run_bass_kernel_spmd(nc, in_maps, list(range(N_CORES)), trace=trace,
                               **spmd_kwargs)
    return gather_output_general(res.results), res


def kernel(**inputs):
    out, _ = run(inputs, trace=False)
    return out


# revision 25
# speedup vs baseline: 1.1768x; 1.0048x over previous
"""Trainium2 Bass kernel for a Bayesian MLP (local reparameterization trick).

Reference computation (per sample s of S=10):
    h1 = leaky_relu(x @ W1m + sqrt(x^2 @ W1v + 1e-12) * eps1_s)         [B, 512]
    h2 = leaky_relu(h1a @ W2m + sqrt(h1a^2 @ W2v + 1e-12) * eps2_s)     (h1a = [h1, 1])
    h3 = leaky_relu(h2a @ W3m + sqrt(h2a^2 @ W3v + 1e-12) * eps3_s)
    out = log_softmax(h3a @ W4m + sqrt(h3a^2 @ W4v + 1e-12) * eps4_s)   [B, 10]

Distribution: data-parallel over the batch axis, B=2048 -> 8 cores x 256 rows.
Small variational parameters replicated on every core.

Fast path (used when every a*_scale array is a constant fill, which holds for
the reference setup where scale = 0.1 * ones):
    x^2 @ (c*ones) = c * ||x||^2  -> the whole variance path collapses to a
    rank-1 partition-sum matmul per layer + one sqrt on a [1, fd] row.
  * activations [feat on 128 partitions, (sample,batch) free], bf16 matmuls
  * sigma*eps is accumulated into the mean PSUM via an identity matmul, so a
    single ACT Prelu (with the folded mean-bias) reads PSUM once per tile
  * per-engine balance: DVE does sig*e products and h^2 squares, ACT does
    prelus/sqrts, Pool (gpsimd) does PSUM->SBUF sigma/u4 copies + L1 squares
  * log-softmax deferred to a final phase (exp/ln table loads once)

General path: the original baseline program (full variance matmuls, f32r).
"""

import sys
import os

for _p in ("/opt/trn_rl_repo",):
    if _p not in sys.path and os.path.isdir(_p):
        sys.path.insert(0, _p)

import numpy as np
import ml_dtypes

import concourse.bass as bass
import concourse.bacc as bacc
import concourse.mybir as mybir
from concourse import tile
from concourse.bass_utils import run_bass_kernel_spmd

F32 = mybir.dt.float32
F32R = mybir.dt.float32r
BF16 = mybir.dt.bfloat16
F8E4 = mybir.dt.float8e4
AF = mybir.ActivationFunctionType
ALU = mybir.AluOpType

B, D_IN, H, C, S = 2048, 784, 512, 10, 10
N_CORES = 8
BL = B // N_CORES            # 256 rows per core
KPAD = 896                   # 784 padded to 7*128
K1 = KPAD // 128             # 7 k-chunks for layer 1
KH = H // 128                # 4 k-chunks for hidden layers
FO = H // 128                # 4 output-feature chunks for hidden layers

bf = ml_dtypes.bfloat16


# --------------------------------------------------------------------------
# Fast path: uniform scale arrays -> rank-1 variance
# --------------------------------------------------------------------------

def build_program_fast(c1, c2, c3, c4, bl=BL, n_pairs=S // 2):
    """Per-core program exploiting sigma_l^2 = c_l * (||h||^2 [+ 1]).

    sqrt is computed as exp(0.5*ln(.)) so the whole kernel runs inside the
    single natural_log_exp ACT table set; log-softmax interleaves per pair.
    sigma rows broadcast across partitions via gpsimd partition_broadcast.
    """
    fd = 2 * bl              # 512 free dim per sample-pair
    nc = bacc.Bacc("TRN2", target_bir_lowering=False, debug=False)

    # ---- DRAM I/O (per core) ----
    xT_d = nc.dram_tensor("xT", [K1, 128, bl], F8E4, kind="ExternalInput")
    x2T_d = nc.dram_tensor("x2T", [K1, 128, bl], F8E4, kind="ExternalInput")
    w1m_d = nc.dram_tensor("w1m", [K1, 128, H], F8E4, kind="ExternalInput")
    w2m_d = nc.dram_tensor("w2m", [KH, 128, H], BF16, kind="ExternalInput")
    w3m_d = nc.dram_tensor("w3m", [KH, 128, H], BF16, kind="ExternalInput")
    w4m_d = nc.dram_tensor("w4m", [128, KH * C], BF16, kind="ExternalInput")
    b2mP_d = nc.dram_tensor("b2mP", [128, FO], F32, kind="ExternalInput")
    b3mP_d = nc.dram_tensor("b3mP", [128, FO], F32, kind="ExternalInput")
    b4m_d = nc.dram_tensor("b4m", [1, C], BF16, kind="ExternalInput")
    e1_d = nc.dram_tensor("e1", [n_pairs, FO, 128, fd], BF16, kind="ExternalInput")
    e2_d = nc.dram_tensor("e2", [n_pairs, FO, 128, fd], BF16, kind="ExternalInput")
    e3_d = nc.dram_tensor("e3", [n_pairs, FO, 128, fd], BF16, kind="ExternalInput")
    e4_d = nc.dram_tensor("e4", [n_pairs, C, fd], BF16, kind="ExternalInput")
    eye128_d = nc.dram_tensor("eye128", [128, 128], BF16, kind="ExternalInput")
    eye10_d = nc.dram_tensor("eye10", [C, C], BF16, kind="ExternalInput")
    o_1x10_d = nc.dram_tensor("o_1x10", [1, C], BF16, kind="ExternalInput")
    n_1x10_d = nc.dram_tensor("n_1x10", [1, C], BF16, kind="ExternalInput")
    o_sq_d = nc.dram_tensor("o_sq", [128, 128], BF16, kind="ExternalInput")
    o_128x10_d = nc.dram_tensor("o_128x10", [128, C], BF16, kind="ExternalInput")
    o_128x1_d = nc.dram_tensor("o_128x1", [128, 1], BF16, kind="ExternalInput")
    o_10x1_d = nc.dram_tensor("o_10x1", [C, 1], BF16, kind="ExternalInput")
    ones_row_d = nc.dram_tensor("ones_row", [1, fd], BF16, kind="ExternalInput")
    out_d = nc.dram_tensor("out", [n_pairs, C, fd], F32, kind="ExternalOutput")

    mm = nc.tensor.matmul

    with tile.TileContext(nc) as tc:
        with (
            tc.tile_pool(name="wp", bufs=1) as wp,
        ):
            # persistent tiles
            w2m_t = [wp.tile([128, H], BF16, tag=f"w2m{k}", name=f"w2m{k}") for k in range(KH)]
            w3m_t = [wp.tile([128, H], BF16, tag=f"w3m{k}", name=f"w3m{k}") for k in range(KH)]
            w4m_t = wp.tile([128, KH * C], BF16, tag="w4m", name="w4m")
            b2mP_t = wp.tile([128, FO], F32, tag="b2mP", name="b2mP")
            b3mP_t = wp.tile([128, FO], F32, tag="b3mP", name="b3mP")
            b4m_t = wp.tile([1, C], BF16, tag="b4m", name="b4m")
            eye128_t = wp.tile([128, 128], BF16, tag="eye128", name="eye128")
            eye10_t = wp.tile([C, C], BF16, tag="eye10", name="eye10")
            o_1x10 = wp.tile([1, C], BF16, tag="o_1x10", name="o_1x10")
            n_1x10 = wp.tile([1, C], BF16, tag="n_1x10", name="n_1x10")
            o_sq = wp.tile([128, 128], BF16, tag="o_sq", name="o_sq")
            o_128x10 = wp.tile([128, C], BF16, tag="o_128x10", name="o_128x10")
            o_128x1 = wp.tile([128, 1], BF16, tag="o_128x1", name="o_128x1")
            o_10x1 = wp.tile([C, 1], BF16, tag="o_10x1", name="o_10x1")
            ones_row = wp.tile([1, fd], BF16, tag="ones_row", name="ones_row")
            mu1_t = wp.tile([128, FO * bl], BF16, tag="mu1", name="mu1")
            sig1b_t = wp.tile([128, bl], BF16, tag="sig1b", name="sig1b")
            z128_t = wp.tile([128, 1], F32, tag="z128", name="z128")
            zC_t = wp.tile([C, 1], F32, tag="zC", name="zC")
            z1_t = wp.tile([1, 1], F32, tag="z1", name="z1")
            cb_t = {}
            for nm, cv, npart in (("c2", c2, 128), ("c3", c3, 128),
                                  ("c4", c4, C)):
                cb_t[nm] = wp.tile([npart, 1], F32, tag=f"cb_{nm}",
                                   name=f"cb_{nm}")
                nc.vector.memset(cb_t[nm][:], float(cv))
            nc.vector.memset(z128_t[:], 0.0)
            nc.vector.memset(zC_t[:], 0.0)
            nc.vector.memset(z1_t[:], 0.0)

            # phase-A-critical small constants first
            nc.sync.dma_start(o_128x1[:], o_128x1_d[:])
            nc.sync.dma_start(o_sq[:], o_sq_d[:])

            # ---------- Phase A: layer-1 mu / sigma, sample-independent ----------
            with (
                tc.tile_pool(name="ap", bufs=1) as ap,
                tc.tile_pool(name="psA", bufs=1, space="PSUM") as psA,
            ):
                w1m_t = [ap.tile([128, H], F8E4, tag=f"w1m{k}", name=f"w1m{k}") for k in range(K1)]
                xT_t = [ap.tile([128, bl], F8E4, tag=f"xT{k}", name=f"xT{k}") for k in range(K1)]
                x2T_t = [ap.tile([128, bl], F8E4, tag=f"x2T{k}", name=f"x2T{k}") for k in range(K1)]
                o_sq8 = ap.tile([128, 128], F8E4, tag="o_sq8", name="o_sq8")
                nc.vector.memset(o_sq8[:], 1.0)
                # ~3.5us of dummy matmuls on memset tiles: wakes the PE HAM
                # clock gate (idle default is 1.2 GHz; sustained activity
                # unlocks 2.4 GHz) before the DMA-dependent real work lands.
                warm_rhs = ap.tile([128, fd], F8E4, tag="warm", name="warm")
                nc.vector.memset(warm_rhs[:], 0.0)
                warm_ps = psA.tile([128, fd], F32, tag="psA_w", name="psA_w")
                for i in range(9):
                    mm(warm_ps[:], o_sq8[:], warm_rhs[:], start=True, stop=True)
                for k in range(K1):
                    nc.sync.dma_start(x2T_t[k][:], x2T_d[k])
                for k in range(K1):
                    nc.sync.dma_start(xT_t[k][:], xT_d[k])
                    nc.sync.dma_start(w1m_t[k][:], w1m_d[k])

                # sigma1 first (it gates pair-0's L1 chain), then mu fo-major
                # so mu1[fo0] lands early and L1(0) overlaps the rest
                mu_ps = [psA.tile([128, bl], F32, tag=f"psA_mu{fo}",
                                  name=f"psA_mu{fo}") for fo in range(FO)]
                s1_ps = psA.tile([128, bl], F32, tag="psA_s1", name="psA_s1")
                for k in range(K1):
                    mm(s1_ps[:], o_sq8[:], x2T_t[k][:],
                       start=(k == 0), stop=(k == K1 - 1))
                lnv1 = ap.tile([128, bl], F32, tag="lnv1", name="lnv1")
                nc.scalar.activation(lnv1[:], s1_ps[:], AF.Ln, scale=float(c1),
                                     bias=z128_t[:])
                nc.scalar.activation(sig1b_t[:], lnv1[:], AF.Exp, scale=0.5,
                                     bias=z128_t[:])
                for fo in range(FO):
                    for k in range(K1):
                        mm(mu_ps[fo][:], w1m_t[k][:, fo * 128:(fo + 1) * 128],
                           xT_t[k][:], start=(k == 0), stop=(k == K1 - 1))
                    nc.vector.tensor_scalar_mul(
                        mu1_t[:, fo * bl:(fo + 1) * bl], mu_ps[fo][:],
                        1.0 / 16.0)

            # ---------- Phase B: per sample-pair, layers 1-4 + softmax ----------
            with (
                tc.tile_pool(name="ep", bufs=3) as ep,
                tc.tile_pool(name="hp", bufs=2) as hp,
                tc.tile_pool(name="tp", bufs=8) as tp,
                tc.tile_pool(name="sgp", bufs=2) as sgp,
                tc.tile_pool(name="psU", bufs=1, space="PSUM") as psU,
                tc.tile_pool(name="psV", bufs=1, space="PSUM") as psV,
            ):
                def dma_eps(p, with_weights=None):
                    e_t = {}
                    for nm, e_d in (("e1", e1_d), ("e2", e2_d), ("e3", e3_d)):
                        e_t[nm] = [ep.tile([128, fd], BF16, tag=f"{nm}_{k}",
                                           name=f"{nm}_{k}") for k in range(FO)]
                        for k in range(FO):
                            nc.sync.dma_start(e_t[nm][k][:], e_d[p, k])
                    e_t["e4"] = ep.tile([C, fd], BF16, tag="e4", name="e4")
                    nc.sync.dma_start(e_t["e4"][:], e4_d[p])
                    return e_t

                def emit_L1(eps):
                    """u1 = mu1 + sig1*e1 entirely in SBUF (no PE)."""
                    h_t, hq_t = [], []
                    for fo in range(FO):
                        sl = slice(fo * bl, (fo + 1) * bl)
                        sig_b = (sig1b_t[:].unsqueeze(1)
                                 .broadcast_to((128, 2, bl)))
                        mu_b = (mu1_t[:, sl].unsqueeze(1)
                                .broadcast_to((128, 2, bl)))
                        t_t = tp.tile([128, fd], BF16, tag="t1", name="t1", bufs=4)
                        nc.vector.tensor_tensor(
                            t_t[:].rearrange("p (s n) -> p s n", s=2),
                            eps["e1"][fo][:].rearrange("p (s n) -> p s n", s=2),
                            sig_b, ALU.mult)
                        u_t = tp.tile([128, fd], BF16, tag="u1", name="u1", bufs=4)
                        nc.vector.tensor_tensor(
                            u_t[:].rearrange("p (s n) -> p s n", s=2),
                            t_t[:].rearrange("p (s n) -> p s n", s=2),
                            mu_b, ALU.add)
                        h = hp.tile([128, fd], BF16, tag=f"h1_{fo}", name=f"h1_{fo}")
                        nc.scalar.activation(h[:], u_t[:], AF.Prelu,
                                             bias=z128_t[:], alpha=0.01)
                        hq = hp.tile([128, fd], BF16, tag=f"h1q_{fo}",
                                     name=f"h1q_{fo}")
                        if fo % 2 == 0:
                            nc.vector.tensor_tensor(hq[:], h[:], h[:], ALU.mult)
                        else:
                            nc.gpsimd.tensor_mul(hq[:], h[:], h[:])
                        h_t.append(h)
                        hq_t.append(hq)
                    return h_t, hq_t

                def emit_varones(hq_t, lhs, npart):
                    """||h||^2 summed over partitions AND broadcast to npart
                    partitions in one go (all-ones stationary matrix)."""
                    v_ps = psV.tile([128, fd], F32, tag="var", name="var", bufs=2)
                    for k in range(KH):
                        mm(v_ps[0:npart, :], lhs[:, 0:npart], hq_t[k][:],
                           start=(k == 0), stop=(k == KH - 1))
                    return v_ps

                def sigma_tail(v_ps, c, cb, npart, tag):
                    """sigma = exp(0.5*ln(c*S + c)) on the broadcast tile."""
                    lnv = tp.tile([npart, fd], F32, tag=f"lnv{tag}",
                                  name=f"lnv{tag}", bufs=2)
                    nc.scalar.activation(lnv[:], v_ps[0:npart, :], AF.Ln,
                                         scale=float(c), bias=cb[:])
                    sigb = sgp.tile([npart, fd], BF16, tag=f"sigb{tag}",
                                    name=f"sigb{tag}")
                    zb = z128_t if npart == 128 else zC_t
                    nc.scalar.activation(sigb[:], lnv[:], AF.Exp,
                                         scale=0.5, bias=zb[:])
                    return sigb

                def emit_hidden(eps_l, hin, sigb, wm_t, bmP_t, htag, vnext):
                    """One hidden layer; accumulates next layer's ||h||^2 into
                    vnext as each hq chunk completes (keeps the sigma chain of
                    the NEXT layer off the PE critical path)."""
                    hout, houtq = [], []
                    for fo in range(FO):
                        u_ps = psU.tile([128, fd], F32, tag=f"u{fo}", name=f"u{fo}")
                        for k in range(KH):
                            mm(u_ps[:], wm_t[k][:, fo * 128:(fo + 1) * 128],
                               hin[k][:], start=(k == 0), stop=False)
                        t_t = tp.tile([128, fd], BF16, tag="t", name="t", bufs=4)
                        nc.vector.tensor_tensor(t_t[:], eps_l[fo][:], sigb[:],
                                                ALU.mult)
                        mm(u_ps[:], eye128_t[:], t_t[:], start=False, stop=True)
                        h = hp.tile([128, fd], BF16, tag=f"{htag}_{fo}",
                                    name=f"{htag}_{fo}")
                        nc.scalar.activation(h[:], u_ps[:], AF.Prelu,
                                             bias=bmP_t[:, fo:fo + 1], alpha=0.01)
                        hq = hp.tile([128, fd], BF16, tag=f"{htag}q_{fo}",
                                     name=f"{htag}q_{fo}")
                        nc.vector.tensor_tensor(hq[:], h[:], h[:], ALU.mult)
                        if vnext is not None:
                            vn, vlhs, vnp = vnext
                            mm(vn[0:vnp, :], vlhs[:, 0:vnp], hq[:],
                               start=(fo == 0), stop=(fo == FO - 1))
                        hout.append(h)
                        houtq.append(hq)
                    return hout, houtq

                def emit_t4(sigb4, eps):
                    t4 = tp.tile([C, fd], BF16, tag="t4", name="t4", bufs=2)
                    nc.vector.tensor_tensor(t4[:], eps["e4"][:], sigb4[:], ALU.mult)
                    return t4

                def emit_L4(p, h3_t, t4):
                    u4_ps = psU.tile([C, fd], F32, tag="u4p", name="u4p")
                    for k in range(KH):
                        mm(u4_ps[:], w4m_t[:, k * C:(k + 1) * C], h3_t[k][:],
                           start=(k == 0), stop=False)
                    mm(u4_ps[:], b4m_t[:], ones_row[:], start=False, stop=False)
                    mm(u4_ps[:], eye10_t[:], t4[:], start=False, stop=True)
                    return u4_ps

                def emit_softmax(p, u4_ps):
                    # log-softmax in place on the u4 PSUM bank
                    e_t = tp.tile([C, fd], BF16, tag="expt", name="expt", bufs=2)
                    nc.scalar.activation(e_t[:], u4_ps[:], AF.Exp, bias=zC_t[:])
                    s_ps = psV.tile([128, fd], F32, tag="var", name="var", bufs=2)
                    mm(s_ps[0:1, :], o_10x1[:], e_t[:], start=True, stop=True)
                    lse_t = tp.tile([1, fd], BF16, tag="lse", name="lse", bufs=2)
                    nc.scalar.activation(lse_t[:], s_ps[0:1, :], AF.Ln,
                                         bias=z1_t[:])
                    mm(u4_ps[:], n_1x10[:], lse_t[:], start=False, stop=True,
                       skip_group_check=True)
                    o_t = tp.tile([C, fd], F32, tag="oct", name="oct", bufs=2)
                    nc.vector.tensor_copy(o_t[:], u4_ps[:])
                    nc.sync.dma_start(out_d[p], o_t[:])

                # prologue: eps(0) first in the DMA queue, then weights, eps(1)
                eps_cur = dma_eps(0)
                for k in range(KH):
                    nc.sync.dma_start(w2m_t[k][:], w2m_d[k])
                nc.sync.dma_start(b2mP_t[:], b2mP_d[:])
                nc.sync.dma_start(eye128_t[:], eye128_d[:])
                eps_next = dma_eps(1) if n_pairs > 1 else None
                for k in range(KH):
                    nc.sync.dma_start(w3m_t[k][:], w3m_d[k])
                nc.sync.dma_start(b3mP_t[:], b3mP_d[:])
                nc.sync.dma_start(w4m_t[:], w4m_d[:])
                nc.sync.dma_start(b4m_t[:], b4m_d[:])
                nc.sync.dma_start(eye10_t[:], eye10_d[:])
                nc.sync.dma_start(o_1x10[:], o_1x10_d[:])
                nc.sync.dma_start(n_1x10[:], n_1x10_d[:])
                nc.sync.dma_start(o_10x1[:], o_10x1_d[:])
                nc.sync.dma_start(o_128x10[:], o_128x10_d[:])
                nc.sync.dma_start(ones_row[:], ones_row_d[:])

                h1_cur = emit_L1(eps_cur)
                sigb2_cur = sigma_tail(emit_varones(h1_cur[1], o_sq, 128), c2,
                                       cb_t["c2"], 128, "2")
                u4_prev = None
                for p in range(n_pairs):
                    h1_t, hq1_t = h1_cur
                    v3_ps = psV.tile([128, fd], F32, tag="var", name="var", bufs=2)
                    h2_t, hq2_t = emit_hidden(eps_cur["e2"], h1_t, sigb2_cur,
                                              w2m_t, b2mP_t, "h2",
                                              (v3_ps, o_sq, 128))
                    sigb3 = sigma_tail(v3_ps, c3, cb_t["c3"], 128, "3")
                    eps_pf = None
                    if p + 2 < n_pairs:
                        eps_pf = dma_eps(p + 2)
                    if u4_prev is not None:
                        emit_softmax(p - 1, u4_prev)
                    v4_ps = psV.tile([128, fd], F32, tag="var", name="var", bufs=2)
                    h3_t, hq3_t = emit_hidden(eps_cur["e3"], h2_t, sigb3,
                                              w3m_t, b3mP_t, "h3",
                                              (v4_ps, o_128x10, C))
                    sigb4 = sigma_tail(v4_ps, c4, cb_t["c4"], C, "4")
                    # t4 before L1(p+1) so L4's eye-matmul never queues behind
                    # the next pair's elementwise block on the DVE
                    t4 = emit_t4(sigb4, eps_cur)
                    h1_next = emit_L1(eps_next) if eps_next is not None else None
                    u4_prev = emit_L4(p, h3_t, t4)
                    if h1_next is not None:
                        sigb2_cur = sigma_tail(emit_varones(h1_next[1], o_sq,
                                                            128), c2,
                                               cb_t["c2"], 128, "2")
                    h1_cur = h1_next
                    eps_cur, eps_next = eps_next, eps_pf
                emit_softmax(n_pairs - 1, u4_prev)

    import concourse.bacc as _bacc_mod
    _orig_gat = _bacc_mod.get_activation_tables

    def _pinned_tables(arch):
        tabs = _orig_gat(arch)
        keep = "natural_log_exp_and_others"
        return {nm: (fns if nm == keep else set()) for nm, fns in tabs.items()}

    _bacc_mod.get_activation_tables = _pinned_tables
    try:
        nc.compile()
    finally:
        _bacc_mod.get_activation_tables = _orig_gat
    return nc


def prepare_core_inputs_fast(inputs, bl=BL, n_pairs=S // 2):
    ns = 2 * n_pairs
    fd = 2 * bl
    f = np.float32
    x = np.asarray(inputs["inputs"], dtype=f)

    def padK(a):
        out = np.zeros((KPAD, a.shape[1]), dtype=f)
        out[:D_IN] = a
        return out

    f8 = ml_dtypes.float8_e4m3fn
    w1m = (padK(np.asarray(inputs["a1_mean"], f)) * np.float32(16.0)) \
        .reshape(K1, 128, H).astype(f8)

    def hidden_w(mean):
        m = np.asarray(mean, f)
        wm = np.ascontiguousarray(m[:H].reshape(KH, 128, H).astype(bf))
        bmP = np.ascontiguousarray(m[H].reshape(FO, 128).T.astype(f))
        return wm, bmP

    w2m, b2mP = hidden_w(inputs["a2_mean"])
    w3m, b3mP = hidden_w(inputs["a3_mean"])

    m4 = np.asarray(inputs["a4_mean"], f)
    w4m = np.ascontiguousarray(m4[:H].reshape(KH, 128, C).transpose(1, 0, 2)
                               .reshape(128, KH * C).astype(bf))
    b4m = np.ascontiguousarray(m4[H].reshape(1, C).astype(bf))

    shared = dict(
        w1m=w1m, w2m=w2m, w3m=w3m, w4m=w4m,
        b2mP=b2mP, b3mP=b3mP, b4m=b4m,
        eye128=np.eye(128, dtype=bf),
        eye10=np.eye(C, dtype=bf),
        o_1x10=np.ones((1, C), dtype=bf),
        o_128x1=np.ones((128, 1), dtype=bf),
        o_sq=np.ones((128, 128), dtype=bf),
        o_128x10=np.ones((128, C), dtype=bf),
        n_1x10=np.full((1, C), -1.0, dtype=bf),
        o_10x1=np.ones((C, 1), dtype=bf),
        ones_row=np.ones((1, fd), dtype=bf),
    )

    eps1 = np.asarray(inputs["eps1"], f)
    eps2 = np.asarray(inputs["eps2"], f)
    eps3 = np.asarray(inputs["eps3"], f)
    eps4 = np.asarray(inputs["eps4"], f)

    def eT(e, b0):
        ec = e[:ns, b0:b0 + bl, :]
        return np.ascontiguousarray(
            ec.reshape(n_pairs, 2, bl, FO, 128).transpose(0, 3, 4, 1, 2)
            .reshape(n_pairs, FO, 128, fd).astype(bf))

    def e4T(e, b0):
        ec = e[:ns, b0:b0 + bl, :]
        return np.ascontiguousarray(
            ec.reshape(n_pairs, 2, bl, C).transpose(0, 3, 1, 2)
            .reshape(n_pairs, C, fd).astype(bf))

    in_maps = []
    for i in range(N_CORES):
        b0 = i * bl
        xT = np.zeros((KPAD, bl), dtype=f)
        xT[:D_IN] = x[b0:b0 + bl].T
        f8 = ml_dtypes.float8_e4m3fn
        m = dict(shared)
        m["xT"] = np.ascontiguousarray(xT.reshape(K1, 128, bl).astype(f8))
        m["x2T"] = np.ascontiguousarray((xT * xT).reshape(K1, 128, bl).astype(f8))
        m["e1"] = eT(eps1, b0)
        m["e2"] = eT(eps2, b0)
        m["e3"] = eT(eps3, b0)
        m["e4"] = e4T(eps4, b0)
        in_maps.append(m)
    return in_maps


def gather_output_fast(results, bl=BL, n_pairs=S // 2):
    ns = 2 * n_pairs
    out = np.empty((ns, N_CORES * bl, C), dtype=np.float32)
    for i, r in enumerate(results):
        oc = np.asarray(r["out"])  # [n_pairs, C, fd]
        oc = oc.reshape(n_pairs, C, 2, bl).transpose(0, 2, 3, 1).reshape(ns, bl, C)
        out[:, i * bl:(i + 1) * bl, :] = oc
    return out


def _uniform_scales(inputs):
    """Return (c1, c2, c3, c4) if every scale array is a constant fill."""
    cs = []
    for nm, drop in (("a1_scale", "a1_dropout"), ("a2_scale", "a2_dropout"),
                     ("a3_scale", "a3_dropout"), ("a4_scale", None)):
        s = np.asarray(inputs[nm], np.float32)
        if s.size == 0 or float(s.max()) != float(s.min()):
            return None
        d = float(np.asarray(inputs[drop], np.float32)) if drop else 1.0
        v = d * float(s.flat[0])
        cs.append(v * v)
    return tuple(cs)


# --------------------------------------------------------------------------
# General fallback path (baseline program, arbitrary scale arrays)
# --------------------------------------------------------------------------

def build_program_general(bl=BL, n_pairs=S // 2, act_lrelu=True):
    """Build the per-core Bass program. All cores run the same program (SPMD)."""
    fd = 2 * bl              # free dim per sample-pair
    nc = bacc.Bacc("TRN2", target_bir_lowering=False, debug=False)

    # ---- DRAM I/O (per core) ----
    xT_d = nc.dram_tensor("xT", [K1, 128, bl], F32R, kind="ExternalInput")
    x2T_d = nc.dram_tensor("x2T", [K1, 128, bl], BF16, kind="ExternalInput")
    w1m_d = nc.dram_tensor("w1m", [K1, 128, H], F32R, kind="ExternalInput")
    w1v_d = nc.dram_tensor("w1v", [K1, 128, H], BF16, kind="ExternalInput")
    w2m_d = nc.dram_tensor("w2m", [KH, 128, H], F32R, kind="ExternalInput")
    w2v_d = nc.dram_tensor("w2v", [KH, 128, H], BF16, kind="ExternalInput")
    w3m_d = nc.dram_tensor("w3m", [KH, 128, H], F32R, kind="ExternalInput")
    w3v_d = nc.dram_tensor("w3v", [KH, 128, H], BF16, kind="ExternalInput")
    w4m_d = nc.dram_tensor("w4m", [128, KH * C], F32R, kind="ExternalInput")
    w4v_d = nc.dram_tensor("w4v", [128, KH * C], BF16, kind="ExternalInput")
    b2m_d = nc.dram_tensor("b2m", [1, H], F32R, kind="ExternalInput")
    b3m_d = nc.dram_tensor("b3m", [1, H], F32R, kind="ExternalInput")
    b4m_d = nc.dram_tensor("b4m", [1, C], F32R, kind="ExternalInput")
    b2v_d = nc.dram_tensor("b2v", [128, FO], F32, kind="ExternalInput")
    b3v_d = nc.dram_tensor("b3v", [128, FO], F32, kind="ExternalInput")
    b4v_d = nc.dram_tensor("b4v", [C, 1], F32, kind="ExternalInput")
    e1_d = nc.dram_tensor("e1", [n_pairs, FO, 128, fd], BF16, kind="ExternalInput")
    e2_d = nc.dram_tensor("e2", [n_pairs, FO, 128, fd], BF16, kind="ExternalInput")
    e3_d = nc.dram_tensor("e3", [n_pairs, FO, 128, fd], BF16, kind="ExternalInput")
    e4_d = nc.dram_tensor("e4", [n_pairs, C, fd], F32, kind="ExternalInput")
    b2mP_d = nc.dram_tensor("b2mP", [128, FO], F32, kind="ExternalInput")
    b3mP_d = nc.dram_tensor("b3mP", [128, FO], F32, kind="ExternalInput")
    ones_row_d = nc.dram_tensor("ones_row_in", [1, fd], F32R, kind="ExternalInput")
    ones10_d = nc.dram_tensor("ones10_in", [C, 1], F32R, kind="ExternalInput")
    out_d = nc.dram_tensor("out", [C, n_pairs * fd], F32, kind="ExternalOutput")

    def mm(out_ap, lhsT_ap, rhs_ap, start, stop):
        nc.tensor.matmul(out_ap, lhsT_ap, rhs_ap, start=start, stop=stop)

    with tile.TileContext(nc) as tc:
        with (
            tc.tile_pool(name="wp", bufs=1) as wp,        # persistent weights
            tc.tile_pool(name="sp", bufs=1) as sp,        # persistent activations
        ):
            # persistent weight tiles
            w2m_t = [wp.tile([128, H], F32R, tag=f"w2m{k}", name=f"w2m{k}") for k in range(KH)]
            w2v_t = [wp.tile([128, H], BF16, tag=f"w2v{k}", name=f"w2v{k}") for k in range(KH)]
            w3m_t = [wp.tile([128, H], F32R, tag=f"w3m{k}", name=f"w3m{k}") for k in range(KH)]
            w3v_t = [wp.tile([128, H], BF16, tag=f"w3v{k}", name=f"w3v{k}") for k in range(KH)]
            w4m_t = wp.tile([128, KH * C], F32R, tag="w4m", name="w4m")
            w4v_t = wp.tile([128, KH * C], BF16, tag="w4v", name="w4v")
            b2m_t = wp.tile([1, H], F32R, tag="b2m", name="b2m")
            b3m_t = wp.tile([1, H], F32R, tag="b3m", name="b3m")
            b4m_t = wp.tile([1, C], F32R, tag="b4m", name="b4m")
            b2v_t = wp.tile([128, FO], F32, tag="b2v", name="b2v")
            b3v_t = wp.tile([128, FO], F32, tag="b3v", name="b3v")
            b4v_t = wp.tile([C, 1], F32, tag="b4v", name="b4v")
            ones_row = wp.tile([1, fd], F32R, tag="ones_row", name="ones_row")
            ones10 = wp.tile([C, 1], F32R, tag="ones10", name="ones10")
            b2mP_t = wp.tile([128, FO], F32, tag="b2mP", name="b2mP")
            b3mP_t = wp.tile([128, FO], F32, tag="b3mP", name="b3mP")
            eps12_t = wp.tile([128, 1], F32, tag="eps12", name="eps12")
            z128_t = wp.tile([128, 1], F32, tag="z128", name="z128")
            zC_t = wp.tile([C, 1], F32, tag="zC", name="zC")
            z1_t = wp.tile([1, 1], F32, tag="z1", name="z1")
            nc.vector.memset(eps12_t[:], 1e-12)
            nc.vector.memset(z128_t[:], 0.0)
            nc.vector.memset(zC_t[:], 0.0)
            nc.vector.memset(z1_t[:], 0.0)

            # persistent per-core activations: mu1/sig1 (shared by all samples)
            mu1_t = sp.tile([128, FO * bl], F32, tag="mu1", name="mu1")
            sig1_t = sp.tile([128, FO * bl], F32, tag="sig1", name="sig1")
            u4_all = sp.tile([C, n_pairs * fd], F32, tag="u4", name="u4")
            out_all = sp.tile([C, n_pairs * fd], F32, tag="outall", name="outall")

            # ---------- Phase A: layer-1 mean/std, sample-independent ----------
            with (
                tc.tile_pool(name="ap", bufs=1) as ap,
                tc.tile_pool(name="psA", bufs=4, space="PSUM") as psA,
            ):
                w1m_t = [ap.tile([128, H], F32R, tag=f"w1m{k}", name=f"w1m{k}") for k in range(K1)]
                w1v_t = [ap.tile([128, H], BF16, tag=f"w1v{k}", name=f"w1v{k}") for k in range(K1)]
                xT_t = [ap.tile([128, bl], F32R, tag=f"xT{k}", name=f"xT{k}") for k in range(K1)]
                x2T_t = [ap.tile([128, bl], BF16, tag=f"x2T{k}", name=f"x2T{k}") for k in range(K1)]
                for k in range(K1):
                    nc.sync.dma_start(w1m_t[k][:], w1m_d[k])
                    nc.sync.dma_start(w1v_t[k][:], w1v_d[k])
                    nc.sync.dma_start(xT_t[k][:], xT_d[k])
                    nc.sync.dma_start(x2T_t[k][:], x2T_d[k])
                for fo in range(FO):
                    mu_ps = psA.tile([128, bl], F32, tag="psA_mu", name="psA_mu")
                    var_ps = psA.tile([128, bl], F32, tag="psA_var", name="psA_var")
                    for k in range(K1):
                        mm(mu_ps[:], w1m_t[k][:, fo * 128:(fo + 1) * 128],
                           xT_t[k][:], start=(k == 0), stop=(k == K1 - 1))
                    for k in range(K1):
                        mm(var_ps[:], w1v_t[k][:, fo * 128:(fo + 1) * 128],
                           x2T_t[k][:], start=(k == 0), stop=(k == K1 - 1))
                    nc.scalar.copy(mu1_t[:, fo * bl:(fo + 1) * bl], mu_ps[:])
                    nc.scalar.activation(sig1_t[:, fo * bl:(fo + 1) * bl],
                                         var_ps[:], AF.Sqrt, bias=eps12_t[:])

            # ---------- Phase B: per sample-pair, layers 1-4 ----------
            with (
                tc.tile_pool(name="ep", bufs=3) as ep,
                tc.tile_pool(name="hp", bufs=2) as hp,
                tc.tile_pool(name="tp", bufs=10) as tp,
                tc.tile_pool(name="psB", bufs=3, space="PSUM") as psB,
                tc.tile_pool(name="ps4", bufs=1, space="PSUM") as ps4,
            ):
                def emit_L1(p):
                    e1_t = [ep.tile([128, fd], BF16, tag=f"e1_{k}", name=f"e1_{k}")
                            for k in range(FO)]
                    for k in range(FO):
                        nc.sync.dma_start(e1_t[k][:], e1_d[p, k])
                    h1_t, h1q_t = [], []
                    for fo in range(FO):
                        sig_b = (sig1_t[:, fo * bl:(fo + 1) * bl]
                                 .unsqueeze(1).broadcast_to((128, 2, bl)))
                        mu_b = (mu1_t[:, fo * bl:(fo + 1) * bl]
                                .unsqueeze(1).broadcast_to((128, 2, bl)))
                        t_t = tp.tile([128, fd], F32, tag="tmp", name="tmp")
                        nc.vector.tensor_tensor(
                            t_t[:].rearrange("p (s n) -> p s n", s=2),
                            e1_t[fo][:].rearrange("p (s n) -> p s n", s=2),
                            sig_b, ALU.mult)
                        u_t = tp.tile([128, fd], F32, tag="tmp", name="tmp")
                        nc.vector.tensor_tensor(
                            u_t[:].rearrange("p (s n) -> p s n", s=2),
                            t_t[:].rearrange("p (s n) -> p s n", s=2),
                            mu_b, ALU.add)
                        h = hp.tile([128, fd], F32R, tag=f"h1_{fo}", name=f"h1_{fo}")
                        nc.scalar.activation(h[:], u_t[:], AF.Prelu,
                                             bias=z128_t[:], alpha=0.01)
                        hq = hp.tile([128, fd], BF16, tag=f"h1q_{fo}", name=f"h1q_{fo}")
                        nc.gpsimd.tensor_mul(hq[:], h[:], h[:])
                        h1_t.append(h)
                        h1q_t.append(hq)
                    return h1_t, h1q_t

                def hidden_layer(p, e_d, hin, hinq, wm_t, wv_t, bm_t, bmP_t, bv_t,
                                 htag):
                    eps_t = [ep.tile([128, fd], BF16, tag=f"{htag}e_{k}",
                                     name=f"{htag}e_{k}") for k in range(FO)]
                    for k in range(FO):
                        nc.sync.dma_start(eps_t[k][:], e_d[p, k])
                    hout, houtq = [], []
                    for fo in range(FO):
                        mu_ps = psB.tile([128, fd], F32, tag="psB_mu", name="psB_mu")
                        var_ps = psB.tile([128, fd], F32, tag="psB_var", name="psB_var")
                        for k in range(KH):
                            mm(mu_ps[:], wm_t[k][:, fo * 128:(fo + 1) * 128],
                               hin[k][:], start=(k == 0), stop=(k == KH - 1))
                        for k in range(KH):
                            mm(var_ps[:], wv_t[k][:, fo * 128:(fo + 1) * 128],
                               hinq[k][:], start=(k == 0), stop=(k == KH - 1))
                        sig_t = tp.tile([128, fd], F32, tag="tmp", name="tmp")
                        nc.scalar.activation(sig_t[:], var_ps[:], AF.Sqrt,
                                             bias=bv_t[:, fo:fo + 1])
                        t_t = tp.tile([128, fd], F32, tag="tmp", name="tmp")
                        nc.vector.tensor_tensor(t_t[:], sig_t[:], eps_t[fo][:],
                                                ALU.mult)
                        u_t = tp.tile([128, fd], F32, tag="tmp", name="tmp")
                        nc.vector.tensor_tensor(u_t[:], t_t[:], mu_ps[:], ALU.add)
                        h = hp.tile([128, fd], F32R, tag=f"{htag}_{fo}",
                                    name=f"{htag}_{fo}")
                        nc.scalar.activation(
                            h[:], u_t[:], AF.Prelu,
                            bias=bmP_t[:, fo:fo + 1], alpha=0.01)
                        hq = hp.tile([128, fd], BF16, tag=f"{htag}q_{fo}",
                                     name=f"{htag}q_{fo}")
                        nc.gpsimd.tensor_mul(hq[:], h[:], h[:])
                        hout.append(h)
                        houtq.append(hq)
                    return hout, houtq

                def emit_L4(p, h3_t, h3q_t):
                    e4_t = ep.tile([C, fd], F32, tag="e4", name="e4")
                    nc.sync.dma_start(e4_t[:], e4_d[p])
                    var4_ps = ps4.tile([C, fd], F32, tag="ps4_var", name="ps4_var")
                    for k in range(KH):
                        mm(var4_ps[:], w4v_t[:, k * C:(k + 1) * C], h3q_t[k][:],
                           start=(k == 0), stop=(k == KH - 1))
                    sig4_t = tp.tile([C, fd], F32, tag="tmp4", name="tmp4", bufs=4)
                    nc.scalar.activation(sig4_t[:], var4_ps[:], AF.Sqrt,
                                         bias=b4v_t[:])
                    t4_t = tp.tile([C, fd], F32, tag="tmp4", name="tmp4", bufs=4)
                    nc.vector.tensor_tensor(t4_t[:], sig4_t[:], e4_t[:], ALU.mult)
                    mu4_ps = ps4.tile([C, fd], F32, tag="ps4_mu", name="ps4_mu")
                    for k in range(KH):
                        mm(mu4_ps[:], w4m_t[:, k * C:(k + 1) * C], h3_t[k][:],
                           start=(k == 0), stop=False)
                    mm(mu4_ps[:], b4m_t[:], ones_row[:], start=False, stop=True)
                    nc.vector.tensor_tensor(u4_all[:, p * fd:(p + 1) * fd],
                                            t4_t[:], mu4_ps[:], ALU.add)

                # software pipeline: L1 of pair p+1 is emitted before the
                # heavy layers of pair p, so PE never idles between pairs
                h1_cur = emit_L1(0)
                for k in range(KH):
                    nc.sync.dma_start(w2m_t[k][:], w2m_d[k])
                    nc.sync.dma_start(w2v_t[k][:], w2v_d[k])
                nc.sync.dma_start(b2m_t[:], b2m_d[:])
                nc.sync.dma_start(b2v_t[:], b2v_d[:])
                nc.sync.dma_start(b2mP_t[:], b2mP_d[:])
                for k in range(KH):
                    nc.sync.dma_start(w3m_t[k][:], w3m_d[k])
                    nc.sync.dma_start(w3v_t[k][:], w3v_d[k])
                nc.sync.dma_start(b3m_t[:], b3m_d[:])
                nc.sync.dma_start(b3v_t[:], b3v_d[:])
                nc.sync.dma_start(b3mP_t[:], b3mP_d[:])
                nc.sync.dma_start(w4m_t[:], w4m_d[:])
                nc.sync.dma_start(w4v_t[:], w4v_d[:])
                nc.sync.dma_start(b4m_t[:], b4m_d[:])
                nc.sync.dma_start(b4v_t[:], b4v_d[:])
                nc.sync.dma_start(ones_row[:], ones_row_d[:])
                nc.sync.dma_start(ones10[:], ones10_d[:])
                for p in range(n_pairs):
                    h1_next = emit_L1(p + 1) if p + 1 < n_pairs else None
                    h1_t, h1q_t = h1_cur
                    h2_t, h2q_t = hidden_layer(p, e2_d, h1_t, h1q_t, w2m_t, w2v_t,
                                               b2m_t, b2mP_t, b2v_t, "h2")
                    h3_t, h3q_t = hidden_layer(p, e3_d, h2_t, h2q_t, w3m_t, w3v_t,
                                               b3m_t, b3mP_t, b3v_t, "h3")
                    emit_L4(p, h3_t, h3q_t)
                    h1_cur = h1_next

            # ---------- Phase C: log-softmax over C (exp/ln table) ----------
            with (
                tc.tile_pool(name="cp", bufs=2) as cp,
                tc.tile_pool(name="psC", bufs=2, space="PSUM") as psC,
            ):
                for p in range(n_pairs):
                    sl = slice(p * fd, (p + 1) * fd)
                    e_t = cp.tile([C, fd], F32R, tag="exp", name="exp")
                    nc.scalar.activation(e_t[:], u4_all[:, sl], AF.Exp, bias=zC_t[:])
                    s_ps = psC.tile([1, fd], F32, tag="psC_s", name="psC_s")
                    mm(s_ps[:], ones10[:], e_t[:], start=True, stop=True)
                    lse_t = cp.tile([1, fd], F32R, tag="lse", name="lse")
                    nc.scalar.activation(lse_t[:], s_ps[:], AF.Ln, bias=z1_t[:])
                    lseb_ps = psC.tile([C, fd], F32, tag="psC_b", name="psC_b")
                    mm(lseb_ps[:], ones_row[0:1, 0:C], lse_t[:], start=True, stop=True)
                    nc.vector.tensor_tensor(out_all[:, sl], u4_all[:, sl],
                                            lseb_ps[:], ALU.subtract)
                nc.sync.dma_start(out_d[:], out_all[:])

    nc.compile()
    return nc


def prepare_core_inputs_general(inputs, bl=BL, n_pairs=S // 2):
    """Host-side preprocessing: shard + transpose + fold parameters."""
    ns = 2 * n_pairs
    fd = 2 * bl
    f = np.float32
    x = np.asarray(inputs["inputs"], dtype=f)

    def padK(a):
        out = np.zeros((KPAD, a.shape[1]), dtype=f)
        out[:D_IN] = a
        return out

    w1m = padK(np.asarray(inputs["a1_mean"], f)).reshape(K1, 128, H)
    s1 = np.asarray(inputs["a1_dropout"], f) * np.asarray(inputs["a1_scale"], f)
    w1v = padK((s1 * s1).astype(f)).reshape(K1, 128, H).astype(bf)

    def hidden_w(mean, scale, dropout):
        m = np.asarray(mean, f)
        sc = (np.asarray(dropout, f) * np.asarray(scale, f)).astype(f)
        v = sc * sc
        wm = np.ascontiguousarray(m[:H].reshape(KH, 128, H))
        wv = np.ascontiguousarray(v[:H].reshape(KH, 128, H).astype(bf))
        bm = np.ascontiguousarray(m[H].reshape(1, H))
        bmP = np.ascontiguousarray(m[H].reshape(FO, 128).T)
        bv = np.ascontiguousarray((v[H] + np.float32(1e-12)).reshape(FO, 128).T)
        return wm, wv, bm, bmP, bv

    w2m, w2v, b2m, b2mP, b2v = hidden_w(inputs["a2_mean"], inputs["a2_scale"],
                                        inputs["a2_dropout"])
    w3m, w3v, b3m, b3mP, b3v = hidden_w(inputs["a3_mean"], inputs["a3_scale"],
                                        inputs["a3_dropout"])

    m4 = np.asarray(inputs["a4_mean"], f)
    s4 = np.asarray(inputs["a4_scale"], f)
    v4 = s4 * s4
    w4m = np.ascontiguousarray(m4[:H].reshape(KH, 128, C).transpose(1, 0, 2)
                               .reshape(128, KH * C))
    w4v = np.ascontiguousarray(v4[:H].reshape(KH, 128, C).transpose(1, 0, 2)
                               .reshape(128, KH * C).astype(bf))
    b4m = np.ascontiguousarray(m4[H].reshape(1, C))
    b4v = np.ascontiguousarray((v4[H] + np.float32(1e-12)).reshape(C, 1))

    shared = dict(w1m=w1m, w1v=w1v, w2m=w2m, w2v=w2v, w3m=w3m, w3v=w3v,
                  w4m=w4m, w4v=w4v, b2m=b2m, b3m=b3m, b4m=b4m,
                  b2mP=b2mP, b3mP=b3mP,
                  b2v=b2v, b3v=b3v, b4v=b4v,
                  ones_row_in=np.ones((1, fd), dtype=f),
                  ones10_in=np.ones((C, 1), dtype=f))

    eps1 = np.asarray(inputs["eps1"], f)
    eps2 = np.asarray(inputs["eps2"], f)
    eps3 = np.asarray(inputs["eps3"], f)
    eps4 = np.asarray(inputs["eps4"], f)

    def eT(e, b0):
        # [ns, bl, H] -> [n_pairs, FO(kchunk), 128, (si, b)]
        ec = e[:ns, b0:b0 + bl, :]
        return np.ascontiguousarray(
            ec.reshape(n_pairs, 2, bl, FO, 128).transpose(0, 3, 4, 1, 2)
            .reshape(n_pairs, FO, 128, fd).astype(ml_dtypes.bfloat16))

    def e4T(e, b0):
        ec = e[:ns, b0:b0 + bl, :]
        return np.ascontiguousarray(
            ec.reshape(n_pairs, 2, bl, C).transpose(0, 3, 1, 2)
            .reshape(n_pairs, C, fd))

    in_maps = []
    for i in range(N_CORES):
        b0 = i * bl
        xT = np.zeros((KPAD, bl), dtype=f)
        xT[:D_IN] = x[b0:b0 + bl].T
        x2T = (xT * xT).astype(ml_dtypes.bfloat16)
        m = dict(shared)
        m["xT"] = np.ascontiguousarray(xT.reshape(K1, 128, bl))
        m["x2T"] = np.ascontiguousarray(x2T.reshape(K1, 128, bl))
        m["e1"] = eT(eps1, b0)
        m["e2"] = eT(eps2, b0)
        m["e3"] = eT(eps3, b0)
        m["e4"] = e4T(eps4, b0)
        in_maps.append(m)
    return in_maps


def gather_output_general(results, bl=BL, n_pairs=S // 2):
    ns = 2 * n_pairs
    out = np.empty((ns, N_CORES * bl, C), dtype=np.float32)
    for i, r in enumerate(results):
        oc = np.asarray(r["out"])  # [C, n_pairs * fd]
        oc = oc.reshape(C, n_pairs, 2, bl).transpose(1, 2, 3, 0).reshape(ns, bl, C)
        out[:, i * bl:(i + 1) * bl, :] = oc
    return out


_CACHE = {}


def run(inputs, trace=False, **spmd_kwargs):
    cs = _uniform_scales(inputs)
    if cs is not None:
        key = ("fast",) + cs
        if key not in _CACHE:
            _CACHE[key] = build_program_fast(*cs)
        nc = _CACHE[key]
        in_maps = prepare_core_inputs_fast(inputs)
        res = run_bass_kernel_spmd(nc, in_maps, list(range(N_CORES)),
                                   trace=trace, **spmd_kwargs)
        return gather_output_fast(res.results), res
    key = ("general",)
    if key not in _CACHE:
        _CACHE[key] = build_program_general()
    nc = _CACHE[key]
    in_maps = prepare_core_inputs_general(inputs)
    res = run_bass_kernel_spmd(nc, in_maps, list(range(N_CORES)), trace=trace,
                               **spmd_kwargs)
    return gather_output_general(res.results), res


def kernel(**inputs):
    out, _ = run(inputs, trace=False)
    return out
